# revision 1
# baseline (speedup 1.0000x reference)
"""BiLSTM-CRF loss kernel for 8 Trainium2 NeuronCores.

Sharding: data-parallel over batch (64 -> 8 per core). Each core runs the
full 3-layer BiLSTM + attention + CRF on its shard and returns the
per-sample (forward_score - gold_score); the host averages them.

On-chip layout: all sequence features are kept transposed as
[feature_dim(partitions), time*batch(free)] so that LSTM gates land on 128
partitions.  Input projections (Wih @ x for every timestep) are precomputed
as large fp16 GEMMs; the sequential recurrence per step is only Whh @ h.
The CRF forward pass runs in probability space (scaled forward algorithm):
one 6x6 matmul + one elementwise multiply per timestep, renormalizing every
RSC steps.
"""

import os
import sys

import numpy as np

sys.path.insert(0, "/opt/trn_rl_repo")

import concourse.bass as bass  # noqa: E402
import concourse.mybir as mybir  # noqa: E402
from concourse import tile  # noqa: E402
from concourse.bass_utils import run_bass_kernel_spmd  # noqa: E402

# ---- problem constants (hardcoded per harness contract) ----
V, E, H, HD, G = 8000, 512, 256, 512, 1024
TAGS, START, STOP = 6, 4, 5
B, T = 64, 256
NCORES = 8
BL = B // NCORES          # local batch = 8
NKE = E // 128            # k-chunks over input features (4)
NKH = H // 128            # k-chunks over hidden (2)
NM = G // 128             # gate chunks (8)
NF = HD // 128            # feature chunks (4)
RSC = 8                   # CRF renormalization interval
TCLIP = 30000.0           # replaces -1e6 in transitions (exp() still == 0)

f32 = mybir.dt.float32
f16 = mybir.dt.float16
i32 = mybir.dt.int32
AF = mybir.ActivationFunctionType
OP = mybir.AluOpType
AX = mybir.AxisListType
PSUM = bass.MemorySpace.PSUM

# gate slot order in PSUM/gates tiles: [i0,i1,f0,f1,o0,o1,g0,g1]
# natural (PyTorch) chunk order in the 4H dim is i(0,1) f(2,3) g(4,5) o(6,7)
SLOT_SRC = [0, 1, 2, 3, 6, 7, 4, 5]
SL_I, SL_F, SL_O, SL_G = slice(0, 2), slice(2, 4), slice(4, 6), slice(6, 8)


def build_program(t_steps=T, reps=1, legalize=True):
    """Build the (single, SPMD-identical) Bass program. Returns nc."""
    TT = t_steps
    TOK = TT * BL
    NGl = (TOK + 127) // 128
    NW = 512 if TOK >= 512 else TOK   # gemm n-tile width
    NN = TOK // NW

    nc = bass.Bass()

    # ---- DRAM I/O ----
    tok_d = nc.dram_tensor("tok_ids", [TOK], i32, kind="ExternalInput")
    tags_d = nc.dram_tensor("tags_tb", [TOK], i32, kind="ExternalInput")
    emb_d = nc.dram_tensor("embed", [V, E], f32, kind="ExternalInput")
    wih_d, whh_d, bias_d = {}, {}, {}
    for l in range(3):
        for d in range(2):
            wih_d[l, d] = nc.dram_tensor(f"wihT_{l}_{d}", [E, G], f16,
                                         kind="ExternalInput")
            whh_d[l, d] = nc.dram_tensor(f"whhT_{l}_{d}", [H, G], f16,
                                         kind="ExternalInput")
            bias_d[l, d] = nc.dram_tensor(f"bias_{l}_{d}", [G], f32,
                                          kind="ExternalInput")
    aw1_d = nc.dram_tensor("aw1T", [HD, HD], f16, kind="ExternalInput")
    ab1_d = nc.dram_tensor("ab1", [HD], f32, kind="ExternalInput")
    aw2_d = nc.dram_tensor("aw2", [HD], f16, kind="ExternalInput")
    hw_d = nc.dram_tensor("hwT", [HD, TAGS], f16, kind="ExternalInput")
    hb_d = nc.dram_tensor("hb", [TAGS], f32, kind="ExternalInput")
    tr_d = nc.dram_tensor("trans", [TAGS, TAGS], f32, kind="ExternalInput")
    trT_d = nc.dram_tensor("transT", [TAGS, TAGS], f32, kind="ExternalInput")
    out_d = nc.dram_tensor("loss_part", [1, BL], f32, kind="ExternalOutput")

    with tile.TileContext(nc) as tc:
        with tc.tile_pool(name="pers", bufs=1) as pers, \
             tc.tile_pool(name="work", bufs=4) as work:

            # ---------- constants / indices ----------
            idx = pers.tile([128, NGl], i32)
            nc.sync.dma_start(idx[:],
                              tok_d[:].rearrange("(g p) -> p g", p=128))
            iop = pers.tile([128, 1], i32)
            nc.gpsimd.iota(iop[:], pattern=[[0, 1]], base=0,
                           channel_multiplier=1)
            iof = pers.tile([128, 128], i32)
            nc.gpsimd.iota(iof[:], pattern=[[1, 128]], base=0,
                           channel_multiplier=0)
            ident = pers.tile([128, 128], f32)
            nc.vector.tensor_tensor(ident[:], iop[:].to_broadcast([128, 128]),
                                    iof[:], op=OP.is_equal)
            identh = pers.tile([128, 128], f16)
            nc.vector.tensor_copy(identh[:], ident[:])
            ones1 = pers.tile([1, 128], f16)
            nc.vector.memset(ones1[:], 1.0)

            # big feature buffers, [128, chunk, tok] fp16
            bufA = pers.tile([128, NF, TOK], f16)
            bufB = pers.tile([128, NF, TOK], f16)

            def emit_pipeline():
                # ---------- phase 1: embedding gather + transpose ----------
                with tc.tile_pool(name="ps_emb", bufs=1, space=PSUM) as ps_emb:
                    for g in range(NGl):
                        gt = work.tile([128, E], f32, tag="gt", bufs=3)
                        nc.gpsimd.indirect_dma_start(
                            out=gt[:], out_offset=None, in_=emb_d[:],
                            in_offset=bass.IndirectOffsetOnAxis(
                                ap=idx[:, g:g + 1], axis=0))
                        for e in range(NKE):
                            tp = ps_emb.tile([128, 128], f32, tag="tp", bufs=4)
                            nc.tensor.transpose(
                                tp[:], gt[:, e * 128:(e + 1) * 128], ident[:])
                            nc.vector.tensor_copy(
                                bufA[:, e, g * 128:(g + 1) * 128], tp[:])

                # ---------- phase 2: LSTM layers ----------
                def lstm_layer(l, in_feat, out_feat, xproj):
                    # --- input projections, both dirs ---
                    # n-loop innermost so the 4 matmuls per (m, k) share one
                    # ldweights (legalization elides repeated loads).
                    with tc.tile_pool(name=f"ps_xp{l}", bufs=1,
                                      space=PSUM) as ps_xp:
                        for d in range(2):
                            wih_t = work.tile([128, NKE, G], f16, tag=f"wih{d}",
                                              bufs=2)
                            nc.sync.dma_start(
                                wih_t[:],
                                wih_d[l, d][:].rearrange("(k p) g -> p k g",
                                                         p=128))
                            bias_t = work.tile([128, NM], f32, tag=f"bias{d}",
                                               bufs=2)
                            nc.sync.dma_start(
                                bias_t[:],
                                bias_d[l, d][:].rearrange("(m p) -> p m", p=128))
                            for m in range(NM):
                                pss = [ps_xp.tile([128, NW], f32,
                                                  name=f"xpps{n}",
                                                  tag=f"gemm{n}", bufs=1)
                                       for n in range(NN)]
                                for k in range(NKE):
                                    for n in range(NN):
                                        nc.tensor.matmul(
                                            pss[n][:],
                                            lhsT=wih_t[:, k,
                                                       m * 128:(m + 1) * 128],
                                            rhs=in_feat[:, k, n * NW:(n + 1) * NW],
                                            start=(k == 0), stop=(k == NKE - 1))
                                for n in range(NN):
                                    nc.scalar.add(
                                        xproj[d][:, m, n * NW:(n + 1) * NW],
                                        pss[n][:], bias_t[:, m:m + 1])
                    # --- recurrence ---
                    # Per step both dirs share one PSUM tile [128, 2, NM, BL]:
                    # an identity matmul accumulates the xproj slice into
                    # PSUM (start=True), the Whh matmuls accumulate on top,
                    # and Sigmoid reads PSUM directly (no DVE add).
                    with tc.tile_pool(name=f"ps_rec{l}", bufs=1,
                                      space=PSUM) as ps_rec:
                        whh_t = {}
                        for d in range(2):
                            whh_t[d] = work.tile([128, NKH, G], f16,
                                                 name=f"whh_t{d}",
                                                 tag=f"whh{d}", bufs=2)
                            nc.sync.dma_start(
                                whh_t[d][:],
                                whh_d[l, d][:].rearrange("(k p) g -> p k g",
                                                         p=128))
                        c_prev = {0: None, 1: None}
                        for step in range(TT):
                            first = step == 0
                            cols = [step * BL, (TT - 1 - step) * BL]
                            # one full PSUM bank (2KB zero region) per dir
                            gps = [ps_rec.tile([128, NM, BL], f32,
                                               name=f"gp{d}", tag=f"gp{d}",
                                               bufs=3) for d in range(2)]
                            # identity matmuls back-to-back share the ldweights
                            for d in range(2):
                                nc.tensor.matmul(
                                    gps[d][:], lhsT=identh[:],
                                    rhs=xproj[d][:, :, cols[d]:cols[d] + BL],
                                    start=True, stop=first)
                            if not first:
                                for d in range(2):
                                    h_col = cols[d] + (BL if d else -BL)
                                    for m in range(NM):
                                        for k in range(NKH):
                                            nc.tensor.matmul(
                                                gps[d][:, m, :],
                                                lhsT=whh_t[d][
                                                    :, k, m * 128:(m + 1) * 128],
                                                rhs=out_feat[:, d * NKH + k,
                                                             h_col:h_col + BL],
                                                start=False,
                                                stop=(m == NM - 1
                                                      and k == NKH - 1))
                            for d in range(2):
                                col = cols[d]
                                sg = work.tile([128, 8, BL], f32, tag=f"sg{d}", bufs=6)
                                nc.scalar.activation(sg[:], gps[d][:], AF.Sigmoid)
                                xx = work.tile([128, 2, BL], f32, tag=f"xx{d}", bufs=6)
                                nc.vector.tensor_tensor(xx[:], sg[:, SL_I, :], sg[:, SL_G, :], op=OP.mult)
                                t1 = work.tile([128, 2, BL], f32, tag=f"t1{d}", bufs=6)
                                nc.vector.scalar_tensor_tensor(out=t1[:], in0=xx[:], scalar=2.0, in1=sg[:, SL_I, :], op0=OP.mult, op1=OP.subtract)
                                if first:
                                    cn = t1
                                else:
                                    cf = work.tile([128, 2, BL], f32,
                                                   tag=f"cf{d}", bufs=6)
                                    nc.vector.tensor_tensor(
                                        cf[:], sg[:, SL_F, :], c_prev[d][:],
                                        op=OP.mult)
                                    cn = work.tile([128, 2, BL], f32,
                                                   tag=f"c{d}", bufs=4)
                                    nc.vector.tensor_tensor(cn[:], t1[:], cf[:],
                                                            op=OP.add)
                                c_prev[d] = cn
                                th = work.tile([128, 2, BL], f32, tag=f"th{d}",
                                               bufs=6)
                                nc.scalar.activation(th[:], cn[:], AF.Tanh)
                                nc.vector.tensor_tensor(
                                    out_feat[:, d * NKH:(d + 1) * NKH,
                                             col:col + BL],
                                    sg[:, SL_O, :], th[:], op=OP.mult)

                with tc.tile_pool(name="xpool", bufs=1) as xpool:
                    xproj = [xpool.tile([128, NM, TOK], f16, name=f"xproj{d}")
                             for d in range(2)]
                    lstm_layer(0, bufA, bufB, xproj)
                    lstm_layer(1, bufB, bufA, xproj)
                    lstm_layer(2, bufA, bufB, xproj)
                L = bufB     # final lstm output [128, NF, TOK] fp16
                tn = bufA    # reuse as tanh/fused buffer

                # ---------- phase 3: attention + emission feats ----------
                with tc.tile_pool(name="apool", bufs=1) as apool, \
                     tc.tile_pool(name="ps_att", bufs=1, space=PSUM) as ps_att:
                    aw1_t = apool.tile([128, NF, HD], f16)
                    nc.sync.dma_start(
                        aw1_t[:], aw1_d[:].rearrange("(k p) g -> p k g", p=128))
                    ab1_t = apool.tile([128, NF], f32)
                    nc.sync.dma_start(
                        ab1_t[:], ab1_d[:].rearrange("(m p) -> p m", p=128))
                    for m in range(NF):
                        for n in range(NN):
                            ps = ps_att.tile([128, NW], f32, tag="gemm", bufs=3)
                            for k in range(NF):
                                nc.tensor.matmul(
                                    ps[:], lhsT=aw1_t[:, k, m * 128:(m + 1) * 128],
                                    rhs=L[:, k, n * NW:(n + 1) * NW],
                                    start=(k == 0), stop=(k == NF - 1))
                            nc.scalar.activation(tn[:, m, n * NW:(n + 1) * NW],
                                                 ps[:], AF.Tanh,
                                                 bias=ab1_t[:, m:m + 1])
                    aw2_t = apool.tile([128, NF], f16)
                    nc.sync.dma_start(aw2_t[:],
                                      aw2_d[:].rearrange("(k p) -> p k", p=128))
                    en = apool.tile([1, TOK], f32)
                    for n in range(NN):
                        eps = ps_att.tile([1, NW], f32, tag="gemm", bufs=3)
                        for k in range(NF):
                            nc.tensor.matmul(eps[:], lhsT=aw2_t[:, k:k + 1],
                                             rhs=tn[:, k, n * NW:(n + 1) * NW],
                                             start=(k == 0), stop=(k == NF - 1))
                        nc.vector.tensor_copy(en[:, n * NW:(n + 1) * NW], eps[:])
                    # softmax over t for each b; en is [1, (t, b)]
                    env = en[:].rearrange("p (t b) -> p b t", b=BL)
                    mx = apool.tile([1, BL], f32)
                    nc.vector.tensor_reduce(mx[:], env, axis=AX.X, op=OP.max)
                    e2 = apool.tile([1, TOK], f32)
                    nc.vector.tensor_tensor(
                        e2[:].rearrange("p (t b) -> p b t", b=BL), env,
                        mx[:].unsqueeze(2).to_broadcast([1, BL, TT]),
                        op=OP.subtract)
                    ex = apool.tile([1, TOK], f32)
                    nc.scalar.activation(ex[:], e2[:], AF.Exp)
                    sm = apool.tile([1, BL], f32)
                    nc.vector.tensor_reduce(
                        sm[:], ex[:].rearrange("p (t b) -> p b t", b=BL),
                        axis=AX.X, op=OP.add)
                    rc = apool.tile([1, BL], f32)
                    nc.vector.reciprocal(rc[:], sm[:])
                    wp1 = apool.tile([1, TOK], f16)
                    nc.vector.tensor_tensor(
                        wp1[:].rearrange("p (t b) -> p b t", b=BL),
                        ex[:].rearrange("p (t b) -> p b t", b=BL),
                        rc[:].unsqueeze(2).to_broadcast([1, BL, TT]),
                        op=OP.mult)
                    nc.vector.tensor_scalar_add(wp1[:], wp1[:], 1.0)
                    for n in range(NN):
                        wb = ps_att.tile([128, NW], f32, tag="gemm", bufs=3)
                        nc.tensor.matmul(wb[:], lhsT=ones1[:],
                                         rhs=wp1[:, n * NW:(n + 1) * NW],
                                         start=True, stop=True)
                        for fc in range(NF):
                            nc.vector.tensor_tensor(
                                tn[:, fc, n * NW:(n + 1) * NW],
                                L[:, fc, n * NW:(n + 1) * NW], wb[:], op=OP.mult)
                    fu = tn  # fused features now live in bufA
                    hw_t = apool.tile([128, NF, TAGS], f16)
                    nc.sync.dma_start(
                        hw_t[:], hw_d[:].rearrange("(k p) t -> p k t", p=128))
                    hb_t = apool.tile([TAGS, 1], f32)
                    nc.sync.dma_start(hb_t[:],
                                      hb_d[:].rearrange("(t a) -> t a", a=1))
                    feats = pers.tile([TAGS, TOK], f32)
                    for n in range(NN):
                        fps = ps_att.tile([TAGS, NW], f32, tag="gemm", bufs=3)
                        for k in range(NF):
                            nc.tensor.matmul(fps[:], lhsT=hw_t[:, k, :],
                                             rhs=fu[:, k, n * NW:(n + 1) * NW],
                                             start=(k == 0), stop=(k == NF - 1))
                        nc.scalar.add(feats[:, n * NW:(n + 1) * NW], fps[:],
                                      hb_t[:])

                # ---------- phase 4: CRF forward (scaled, prob space) ----------
                # Batch-major on partitions: alpha is [BL, TAGS]; one step is
                # two back-to-back DVE ops (mult by precomputed Ae[b,t,j,i],
                # reduce over i) -- no cross-engine ping-pong on the chain.
                with tc.tile_pool(name="cpool", bufs=1) as cpool:
                  with tc.tile_pool(name="ps_crf", bufs=1, space=PSUM) as ps_crf:
                      tr_t = cpool.tile([TAGS, TAGS], f32)
                      nc.sync.dma_start(tr_t[:], tr_d[:])
                      trT_t = cpool.tile([TAGS, TAGS], f32)
                      nc.sync.dma_start(trT_t[:], trT_d[:])
                      iot6 = cpool.tile([TAGS, 1], f32)
                      nc.vector.tensor_copy(iot6[:], iop[0:TAGS, :])
                      ones6 = cpool.tile([TAGS, 1], f32)
                      nc.vector.memset(ones6[:], 1.0)
                      # flat trans (j,i) on one partition; exp; replicate to
                      # BL partitions with a ones-matmul broadcast
                      a1 = cpool.tile([1, TAGS * TAGS], f32)
                      nc.sync.dma_start(a1[:],
                                        tr_d[:].rearrange("j i -> (j i)"))
                      ea1 = cpool.tile([1, TAGS * TAGS + TAGS], f32)
                      nc.scalar.activation(ea1[:, :TAGS * TAGS], a1[:], AF.Exp)
                      nc.scalar.activation(
                          ea1[:, TAGS * TAGS:],
                          a1[:, STOP * TAGS:(STOP + 1) * TAGS], AF.Exp)
                      ones8f = cpool.tile([1, BL], f32)
                      nc.vector.memset(ones8f[:], 1.0)
                      rep_ps = ps_crf.tile([BL, TAGS * TAGS + TAGS], f32,
                                           tag="rep", bufs=1)
                      nc.tensor.matmul(rep_ps[:], lhsT=ones8f[:], rhs=ea1[:],
                                       start=True, stop=True)
                      a8 = cpool.tile([BL, TAGS * TAGS], f32)
                      nc.vector.tensor_copy(a8[:], rep_ps[:, :TAGS * TAGS])
                      es8 = cpool.tile([BL, TAGS], f32)
                      nc.vector.tensor_copy(es8[:], rep_ps[:, TAGS * TAGS:])
                      # exp(feats) then permute (j,(t,b)) -> (b,(t,j))
                      expF = cpool.tile([TAGS, TOK], f32)
                      nc.scalar.activation(expF[:], feats[:], AF.Exp)
                      expT = cpool.tile([BL, TT * TAGS], f32)
                      # permute (j,(t,b)) -> (b,(t,j)) via a DRAM bounce
                      # (partition-crossing SBUF->SBUF APs don't balance)
                      ef_d = nc.dram_tensor(f"ef_scratch{_rep}", [TAGS, TOK],
                                            f32, kind="Internal")
                      nc.sync.dma_start(ef_d[:], expF[:])
                      expT3 = expT[:].rearrange("b (t j) -> b t j", j=TAGS)
                      for j in range(TAGS):
                          nc.sync.dma_start(
                              expT3[:, :, j:j + 1],
                              ef_d[j:j + 1, :].rearrange(
                                  "a (t b) -> b t a", b=BL))
                      # Ae[b,t,j,i] = expT[b,t,j] * exp(trans)[j,i]
                      ae = cpool.tile([BL, TT, TAGS, TAGS], f32)
                      nc.vector.tensor_tensor(
                          ae[:],
                          expT[:].rearrange("b (t j) -> b t j", j=TAGS)
                          .unsqueeze(3).to_broadcast([BL, TT, TAGS, TAGS]),
                          a8[:].rearrange("b (j i) -> b j i", i=TAGS)
                          .unsqueeze(1).to_broadcast([BL, TT, TAGS, TAGS]),
                          op=OP.mult)
                      # alpha0[b,i] = (i == START)
                      iof8 = cpool.tile([BL, TAGS], f32)
                      nc.vector.tensor_copy(iof8[:], iof[0:BL, 0:TAGS])
                      al0 = cpool.tile([BL, TAGS], f32)
                      nc.vector.tensor_scalar(
                          out=al0[:], in0=iof8[:], scalar1=float(START),
                          scalar2=None, op0=OP.is_equal)
                      logz8 = cpool.tile([BL, 1], f32)
                      nc.vector.memset(logz8[:], 0.0)
                      lz = logz8
                      al = al0
                      for t in range(TT):
                          u = work.tile([BL, TAGS, TAGS], f32, tag="crfu",
                                        bufs=4)
                          nc.vector.tensor_tensor(
                              u[:],
                              al[:].unsqueeze(1)
                              .to_broadcast([BL, TAGS, TAGS]),
                              ae[:, t], op=OP.mult)
                          an = work.tile([BL, TAGS], f32, tag="crfa", bufs=4)
                          nc.vector.tensor_reduce(an[:], u[:], axis=AX.X,
                                                  op=OP.add)
                          al = an
                          if t % RSC == RSC - 1 and t != TT - 1:
                              s = work.tile([BL, 1], f32, tag="crfs", bufs=2)
                              nc.vector.tensor_reduce(s[:], al[:], axis=AX.X,
                                                      op=OP.add)
                              rc = work.tile([BL, 1], f32, tag="crfr", bufs=2)
                              nc.vector.reciprocal(rc[:], s[:])
                              al2 = work.tile([BL, TAGS], f32, tag="crfa",
                                              bufs=4)
                              nc.vector.tensor_tensor(
                                  al2[:], al[:],
                                  rc[:].to_broadcast([BL, TAGS]), op=OP.mult)
                              al = al2
                              lg = work.tile([BL, 1], f32, tag="crflg", bufs=2)
                              nc.scalar.activation(lg[:], s[:], AF.Ln)
                              lzn = work.tile([BL, 1], f32, tag="crflz",
                                              bufs=2)
                              nc.vector.tensor_tensor(lzn[:], lz[:], lg[:],
                                                      op=OP.add)
                              lz = lzn
                      fin = work.tile([BL, TAGS], f32, tag="crfu", bufs=4)
                      nc.vector.tensor_tensor(fin[:], al[:], es8[:],
                                              op=OP.mult)
                      fsum = work.tile([BL, 1], f32, tag="crfs", bufs=2)
                      nc.vector.tensor_reduce(fsum[:], fin[:], axis=AX.X,
                                              op=OP.add)
                      fsl = cpool.tile([BL, 1], f32)
                      nc.scalar.activation(fsl[:], fsum[:], AF.Ln)
                      fsb = cpool.tile([BL, 1], f32)
                      nc.vector.tensor_tensor(fsb[:], fsl[:], lz[:], op=OP.add)
                      fs2_ps = ps_crf.tile([1, BL], f32, tag="fs2", bufs=1)
                      nc.tensor.transpose(fs2_ps[:], fsb[:], ident[0:BL, 0:BL])
                      fs2 = pers.tile([1, BL], f32)
                      nc.vector.tensor_copy(fs2[:], fs2_ps[:])

                  # ---------- phase 5: gold path score ----------
                  with tc.tile_pool(name="ps_gold", bufs=1, space=PSUM) as ps_gold:
                      tags_sb = cpool.tile([1, TOK], i32)
                      nc.sync.dma_start(tags_sb[:],
                                        tags_d[:].rearrange("(a t) -> a t", a=1))
                      tagf = cpool.tile([1, TOK], f16)
                      nc.vector.tensor_copy(tagf[:], tags_sb[:])
                      iotf = iot6
                      oh = cpool.tile([TAGS, TOK], f32)
                      for n in range(NN):
                          tb_ps = ps_gold.tile([TAGS, NW], f32, tag="gemm", bufs=2)
                          nc.tensor.matmul(tb_ps[:], lhsT=ones1[:, 0:TAGS],
                                           rhs=tagf[:, n * NW:(n + 1) * NW],
                                           start=True, stop=True)
                          nc.vector.tensor_tensor(
                              oh[:, n * NW:(n + 1) * NW], tb_ps[:],
                              iotf[:].to_broadcast([TAGS, NW]), op=OP.is_equal)
                      tcol = cpool.tile([TAGS, TOK], f32)
                      for n in range(NN):
                          tc_ps = ps_gold.tile([TAGS, NW], f32, tag="gemm", bufs=2)
                          nc.tensor.matmul(tc_ps[:], lhsT=tr_t[:],
                                           rhs=oh[:, n * NW:(n + 1) * NW],
                                           start=True, stop=True)
                          nc.vector.tensor_copy(tcol[:, n * NW:(n + 1) * NW],
                                                tc_ps[:])
                      gg = cpool.tile([TAGS, TOK], f32)
                      nc.vector.tensor_tensor(gg[:], oh[:], feats[:], op=OP.mult)
                      g2 = cpool.tile([TAGS, TOK], f32)
                      nc.vector.tensor_tensor(g2[:, BL:], oh[:, 0:TOK - BL],
                                              tcol[:, BL:], op=OP.mult)
                      nc.vector.tensor_tensor(
                          g2[:, 0:BL], oh[:, 0:BL],
                          tr_t[:, START:START + 1].to_broadcast([TAGS, BL]),
                          op=OP.mult)
                      gl = cpool.tile([TAGS, BL], f32)
                      nc.vector.tensor_tensor(
                          gl[:], oh[:, TOK - BL:],
                          trT_t[:, STOP:STOP + 1].to_broadcast([TAGS, BL]),
                          op=OP.mult)
                      gold_ps = ps_gold.tile([1, TOK], f32, tag="gold", bufs=1)
                      for n in range(NN):
                          nsl = slice(n * NW, (n + 1) * NW)
                          nc.tensor.matmul(gold_ps[:, nsl], lhsT=ones6[:],
                                           rhs=gg[:, nsl], start=True, stop=False)
                          last = (n == NN - 1)
                          nc.tensor.matmul(gold_ps[:, nsl], lhsT=ones6[:],
                                           rhs=g2[:, nsl], start=False,
                                           stop=not last)
                      nc.tensor.matmul(gold_ps[:, TOK - BL:], lhsT=ones6[:],
                                       rhs=gl[:], start=False, stop=True)
                      gold = pers.tile([1, BL], f32)
                      nc.vector.tensor_reduce(
                          gold[:], gold_ps[:].rearrange("p (t b) -> p b t", b=BL),
                          axis=AX.X, op=OP.add)
                      loss = pers.tile([1, BL], f32)
                      nc.vector.tensor_tensor(loss[:], fs2[:], gold[:],
                                              op=OP.subtract)
                      nc.sync.dma_start(out_d[:], loss[:])

            for _rep in range(reps):
                if _rep:
                    tc.strict_bb_all_engine_barrier()
                emit_pipeline()

    if legalize:
        _legalize_multi_waits(nc)
    return nc


_LW_COUNT = [0]


_SELF_SEM = {mybir.EngineType.DVE: "DVE_", mybir.EngineType.Activation: "Activation_"}


def _legalize_multi_waits(nc):
    """This container's walrus accepts at most ONE sync wait per TPB
    instruction ("Too many sync wait commands" in codegen otherwise), while
    Tile freely attaches several.  Split: keep one wait on the instruction
    and hoist the rest onto standalone InstEventSemaphore instructions on
    the same engine immediately before it (engine-order preserved, so the
    semantics are identical; the stall just happens one slot earlier)."""
    n_new = 0
    for fn in nc.m.functions:
        for bb in fn.blocks:
            out = []
            changed = False
            for inst in bb.instructions:
                si = inst.sync_info
                waits = list(si.on_wait) if si is not None else []
                pref = _SELF_SEM.get(inst.engine)
                if pref is not None and len(waits) > 1:
                    # DVE/ACT are strict in-order single pipelines: a wait on
                    # the engine's own completion semaphore is subsumed by
                    # program order. Drop it (only when other waits remain).
                    kept = [w for w in waits
                            if not str(w.ant_name).startswith(pref)]
                    if kept:
                        waits = kept
                        inst.sync_info = mybir.SyncInfo(
                            on_wait=waits, on_update=list(si.on_update))
                        si = inst.sync_info
                        changed = True
                cap = 1
                if len(waits) > cap:
                    for w in waits[:-cap]:
                        _LW_COUNT[0] += 1
                        es = mybir.InstEventSemaphore(
                            name=f"I-lw{_LW_COUNT[0]}", ins=[], outs=[])
                        es.engine = inst.engine
                        es.sync_info = mybir.SyncInfo(on_wait=[w],
                                                      on_update=[])
                        out.append(es)
                        n_new += 1
                    inst.sync_info = mybir.SyncInfo(
                        on_wait=waits[-cap:], on_update=list(si.on_update))
                    changed = True
                out.append(inst)
            if changed:
                bb.instructions = out
    return n_new


_PROG_CACHE = {}


def _get_program(t_steps=T):
    if t_steps not in _PROG_CACHE:
        _PROG_CACHE[t_steps] = build_program(t_steps)
    return _PROG_CACHE[t_steps]


def _slot_reorder_cols(wT):
    """Reorder the 4H (=1024) columns of [in, 4H] into slot order, and
    pre-scale the g-gate slots by 2 (tanh(g) = 2*sigmoid(2g) - 1, so the
    kernel runs a single Sigmoid over all gates)."""
    chunks = [wT[:, c * 128:(c + 1) * 128].copy() for c in SLOT_SRC]
    chunks[6] = chunks[6] * 2.0
    chunks[7] = chunks[7] * 2.0
    return np.ascontiguousarray(np.concatenate(chunks, axis=1))


def _slot_reorder_vec(v):
    chunks = [v[c * 128:(c + 1) * 128].copy() for c in SLOT_SRC]
    chunks[6] = chunks[6] * 2.0
    chunks[7] = chunks[7] * 2.0
    return np.ascontiguousarray(np.concatenate(chunks))


def prep_inputs(sentences, tags, mask, embed, w_ih, w_hh, b_ih, b_hh,
                aw1, ab1, aw2, ab2, hw, hb, transitions, t_steps=T):
    sent = np.asarray(sentences).astype(np.int32)
    tg = np.asarray(tags).astype(np.int32)
    emb = np.ascontiguousarray(np.asarray(embed, dtype=np.float32))
    w_ih = np.asarray(w_ih, dtype=np.float32)
    w_hh = np.asarray(w_hh, dtype=np.float32)
    b_ih = np.asarray(b_ih, dtype=np.float32)
    b_hh = np.asarray(b_hh, dtype=np.float32)
    trans = np.clip(np.asarray(transitions, dtype=np.float32), -TCLIP, TCLIP)

    shared = {"embed": emb,
              "aw1T": np.ascontiguousarray(
                  np.asarray(aw1, np.float32).T.astype(np.float16)),
              "ab1": np.ascontiguousarray(np.asarray(ab1, np.float32)),
              "aw2": np.ascontiguousarray(
                  np.asarray(aw2, np.float32).astype(np.float16)),
              "hwT": np.ascontiguousarray(
                  np.asarray(hw, np.float32).T.astype(np.float16)),
              "hb": np.ascontiguousarray(np.asarray(hb, np.float32)),
              "trans": np.ascontiguousarray(trans),
              "transT": np.ascontiguousarray(trans.T)}
    for l in range(3):
        for d in range(2):
            shared[f"wihT_{l}_{d}"] = _slot_reorder_cols(
                w_ih[l, d].T).astype(np.float16)
            shared[f"whhT_{l}_{d}"] = _slot_reorder_cols(
                w_hh[l, d].T).astype(np.float16)
            shared[f"bias_{l}_{d}"] = _slot_reorder_vec(
                b_ih[l, d] + b_hh[l, d])

    in_maps = []
    for c in range(NCORES):
        sh = slice(c * BL, (c + 1) * BL)
        m = dict(shared)
        # (t, b) order, b innermost
        m["tok_ids"] = np.ascontiguousarray(
            sent[sh, :t_steps].T).reshape(-1)
        m["tags_tb"] = np.ascontiguousarray(tg[sh, :t_steps].T).reshape(-1)
        in_maps.append(m)
    return in_maps


def kernel(sentences, tags, mask, embed, w_ih, w_hh, b_ih, b_hh,
           aw1, ab1, aw2, ab2, hw, hb, transitions, _trace=False):
    nc = _get_program()
    in_maps = prep_inputs(sentences, tags, mask, embed, w_ih, w_hh,
                          b_ih, b_hh, aw1, ab1, aw2, ab2, hw, hb,
                          transitions)
    res = run_bass_kernel_spmd(nc, in_maps, core_ids=list(range(NCORES)),
                               trace=_trace)
    parts = np.concatenate([r["loss_part"].reshape(-1) for r in res.results])
    kernel.last_result = res
    return np.float32(parts.mean())



# revision 6
# speedup vs baseline: 1.0035x; 1.0035x over previous
"""BiLSTM-CRF loss kernel for 8 Trainium2 NeuronCores.

Sharding: data-parallel over batch (64 -> 8 per core). Each core runs the
full 3-layer BiLSTM + attention + CRF on its shard and returns the
per-sample (forward_score - gold_score); the host averages them.

On-chip layout: all sequence features are kept transposed as
[feature_dim(partitions), time*batch(free)] so that LSTM gates land on 128
partitions.  Input projections (Wih @ x for every timestep) are computed
as fp16 GEMMs interleaved just-in-time into the recurrence's PE idle gaps
(window-ahead emission).  The sequential recurrence per step is Whh @ h
with Whh stored fp8(e4m3) -- LDWEIGHTS is the dominant PE cost at free
dim BL=8 and fp8 weight loads are ~2x faster (FWL); h stays fp16.
The per-step tail is split by h-chunk (k-major matmuls + split tanh/mult)
so the next step's k0 matmuls start as soon as chunk0 of h is ready.
The CRF forward pass runs in probability space (scaled forward algorithm).
"""

import os
import sys

import numpy as np
import ml_dtypes

sys.path.insert(0, "/opt/trn_rl_repo")

import concourse.bass as bass  # noqa: E402
import concourse.mybir as mybir  # noqa: E402
from concourse import tile  # noqa: E402
from concourse.bass_utils import run_bass_kernel_spmd  # noqa: E402

# ---- problem constants (hardcoded per harness contract) ----
V, E, H, HD, G = 8000, 512, 256, 512, 1024
TAGS, START, STOP = 6, 4, 5
B, T = 64, 256
NCORES = 8
BL = B // NCORES          # local batch = 8
NKE = E // 128            # k-chunks over input features (4)
NKH = H // 128            # k-chunks over hidden (2)
NM = G // 128             # gate chunks (8)
NF = HD // 128            # feature chunks (4)
RSC = 8                   # CRF renormalization interval
TCLIP = 30000.0           # replaces -1e6 in transitions (exp() still == 0)

# dev flags (defaults are the shipped config; env only for experiments)
FP8_WHH = os.environ.get("K_FP8", "1") == "1"
SPLIT_TAIL = os.environ.get("K_SPLIT_TAIL", "1") == "1"
XJIT = os.environ.get("K_XJIT", "1") == "1"

f32 = mybir.dt.float32
f16 = mybir.dt.float16
f8 = mybir.dt.float8e4
i32 = mybir.dt.int32
AF = mybir.ActivationFunctionType
OP = mybir.AluOpType
AX = mybir.AxisListType
PSUM = bass.MemorySpace.PSUM

# gate slot order in PSUM/gates tiles: [i0,i1,f0,f1,o0,o1,g0,g1]
# natural (PyTorch) chunk order in the 4H dim is i(0,1) f(2,3) g(4,5) o(6,7)
SLOT_SRC = [0, 1, 2, 3, 6, 7, 4, 5]
SL_I, SL_F, SL_O, SL_G = slice(0, 2), slice(2, 4), slice(4, 6), slice(6, 8)

WH_DT = f8 if FP8_WHH else f16


def build_program(t_steps=T, reps=1, legalize=True):
    """Build the (single, SPMD-identical) Bass program. Returns nc."""
    TT = t_steps
    TOK = TT * BL
    NGl = (TOK + 127) // 128
    NW = 512 if TOK >= 512 else TOK   # gemm n-tile width
    NN = TOK // NW
    SN = TT // NN                     # recurrence steps per xproj n-tile

    nc = bass.Bass()

    # ---- DRAM I/O ----
    tok_d = nc.dram_tensor("tok_ids", [TOK], i32, kind="ExternalInput")
    tags_d = nc.dram_tensor("tags_tb", [TOK], i32, kind="ExternalInput")
    emb_d = nc.dram_tensor("embed", [V, E], f32, kind="ExternalInput")
    wih_d, whh_d, bias_d = {}, {}, {}
    for l in range(3):
        for d in range(2):
            wih_d[l, d] = nc.dram_tensor(f"wihT_{l}_{d}", [E, G], f16,
                                         kind="ExternalInput")
            whh_d[l, d] = nc.dram_tensor(f"whhT_{l}_{d}", [H, G], WH_DT,
                                         kind="ExternalInput")
            bias_d[l, d] = nc.dram_tensor(f"bias_{l}_{d}", [G], f32,
                                          kind="ExternalInput")
    id8_d = nc.dram_tensor("ident8", [128, 128], WH_DT, kind="ExternalInput")
    aw1_d = nc.dram_tensor("aw1T", [HD, HD], f16, kind="ExternalInput")
    ab1_d = nc.dram_tensor("ab1", [HD], f32, kind="ExternalInput")
    aw2_d = nc.dram_tensor("aw2", [HD], f16, kind="ExternalInput")
    hw_d = nc.dram_tensor("hwT", [HD, TAGS], f16, kind="ExternalInput")
    hb_d = nc.dram_tensor("hb", [TAGS], f32, kind="ExternalInput")
    tr_d = nc.dram_tensor("trans", [TAGS, TAGS], f32, kind="ExternalInput")
    trT_d = nc.dram_tensor("transT", [TAGS, TAGS], f32, kind="ExternalInput")
    out_d = nc.dram_tensor("loss_part", [1, BL], f32, kind="ExternalOutput")

    with tile.TileContext(nc) as tc:
        with tc.tile_pool(name="pers", bufs=1) as pers, \
             tc.tile_pool(name="work", bufs=4) as work:

            # ---------- constants / indices ----------
            idx = pers.tile([128, NGl], i32)
            nc.sync.dma_start(idx[:],
                              tok_d[:].rearrange("(g p) -> p g", p=128))
            iop = pers.tile([128, 1], i32)
            nc.gpsimd.iota(iop[:], pattern=[[0, 1]], base=0,
                           channel_multiplier=1)
            iof = pers.tile([128, 128], i32)
            nc.gpsimd.iota(iof[:], pattern=[[1, 128]], base=0,
                           channel_multiplier=0)
            ident = pers.tile([128, 128], f32)
            nc.vector.tensor_tensor(ident[:], iop[:].to_broadcast([128, 128]),
                                    iof[:], op=OP.is_equal)
            ident8 = pers.tile([128, 128], WH_DT)
            nc.sync.dma_start(ident8[:], id8_d[:])
            ones1 = pers.tile([1, 128], f16)
            nc.vector.memset(ones1[:], 1.0)

            # big feature buffers, [128, chunk, tok] fp16
            bufA = pers.tile([128, NF, TOK], f16)
            bufB = pers.tile([128, NF, TOK], f16)

            def emit_pipeline():
                # ---------- phase 1: embedding gather + transpose ----------
                with tc.tile_pool(name="ps_emb", bufs=1, space=PSUM) as ps_emb:
                    for g in range(NGl):
                        gt = work.tile([128, E], f32, tag="gt", bufs=3)
                        nc.gpsimd.indirect_dma_start(
                            out=gt[:], out_offset=None, in_=emb_d[:],
                            in_offset=bass.IndirectOffsetOnAxis(
                                ap=idx[:, g:g + 1], axis=0))
                        for e in range(NKE):
                            tp = ps_emb.tile([128, 128], f32, tag="tp", bufs=4)
                            nc.tensor.transpose(
                                tp[:], gt[:, e * 128:(e + 1) * 128], ident[:])
                            nc.vector.tensor_copy(
                                bufA[:, e, g * 128:(g + 1) * 128], tp[:])

                # ---------- phase 2: LSTM layers ----------
                def lstm_layer(l, in_feat, out_feat, xproj):
                    with tc.tile_pool(name=f"ps_l{l}", bufs=1,
                                      space=PSUM) as psl:
                        # --- weight/bias DMA ---
                        wih_t, whh_t, bias_t = {}, {}, {}
                        for d in range(2):
                            wih_t[d] = work.tile([128, NKE, G], f16,
                                                 name=f"wih_t{d}",
                                                 tag=f"wih{d}", bufs=2)
                            nc.sync.dma_start(
                                wih_t[d][:],
                                wih_d[l, d][:].rearrange("(k p) g -> p k g",
                                                         p=128))
                            whh_t[d] = work.tile([128, NKH, G], WH_DT,
                                                 name=f"whh_t{d}",
                                                 tag=f"whh{d}", bufs=2)
                            nc.sync.dma_start(
                                whh_t[d][:],
                                whh_d[l, d][:].rearrange("(k p) g -> p k g",
                                                         p=128))
                            bias_t[d] = work.tile([128, NM], f32,
                                                  name=f"bias_t{d}",
                                                  tag=f"bias{d}", bufs=2)
                            nc.sync.dma_start(
                                bias_t[d][:],
                                bias_d[l, d][:].rearrange("(m p) -> p m",
                                                          p=128))

                        # --- xproj unit emitters (one unit = d, m, n-tile;
                        #     emitted as two k-halves so each PE burst is
                        #     short enough to hide in a recurrence gap) ---
                        xpps_live = {}

                        def emit_half(d, m, n, half):
                            if half == 0:
                                ps = psl.tile([128, NW], f32, name="xpps",
                                              tag="xpps", bufs=2)
                                xpps_live[(d, m, n)] = ps
                            else:
                                ps = xpps_live.pop((d, m, n))
                            for k in (2 * half, 2 * half + 1):
                                nc.tensor.matmul(
                                    ps[:],
                                    lhsT=wih_t[d][:, k, m * 128:(m + 1) * 128],
                                    rhs=in_feat[:, k, n * NW:(n + 1) * NW],
                                    start=(k == 0), stop=(k == NKE - 1))
                            if half == 1:
                                nc.scalar.add(
                                    xproj[d][:, m, n * NW:(n + 1) * NW],
                                    ps[:], bias_t[d][:, m:m + 1])

                        # upfront: first-needed n-tiles for both dirs
                        jit_units = []   # remaining, window-ordered
                        if XJIT and NN > 1:
                            for d, n in ((0, 0), (1, NN - 1)):
                                for m in range(NM):
                                    emit_half(d, m, n, 0)
                                    emit_half(d, m, n, 1)
                            for w in range(NN - 1):
                                wu = []
                                for m in range(NM):
                                    wu.append((0, m, w + 1))
                                    wu.append((1, m, NN - 2 - w))
                                halves = []
                                for u in wu:
                                    halves.append(u + (0,))
                                    halves.append(u + (1,))
                                jit_units.append(halves)
                        else:
                            for d in range(2):
                                for n in range(NN):
                                    for m in range(NM):
                                        emit_half(d, m, n, 0)
                                        emit_half(d, m, n, 1)

                        # --- recurrence ---
                        c_prev = {0: None, 1: None}
                        for step in range(TT):
                            first = step == 0
                            cols = [step * BL, (TT - 1 - step) * BL]
                            for d in range(2):
                                col = cols[d]
                                gp = psl.tile([128, NM, BL], f32,
                                              name=f"gp{d}", tag=f"gp{d}",
                                              bufs=3)
                                nc.tensor.matmul(
                                    gp[:], lhsT=ident8[:],
                                    rhs=xproj[d][:, :, col:col + BL],
                                    start=True, stop=first)
                                if not first:
                                    h_col = col + (BL if d else -BL)
                                    for k in range(NKH):
                                        for m in range(NM):
                                            nc.tensor.matmul(
                                                gp[:, m, :],
                                                lhsT=whh_t[d][
                                                    :, k,
                                                    m * 128:(m + 1) * 128],
                                                rhs=out_feat[:, d * NKH + k,
                                                             h_col:h_col + BL],
                                                start=False,
                                                stop=(k == NKH - 1
                                                      and m == NM - 1))
                                sg = work.tile([128, 8, BL], f32,
                                               tag=f"sg{d}", bufs=6)
                                nc.scalar.activation(sg[:], gp[:], AF.Sigmoid)
                                if not first:
                                    cf = work.tile([128, 2, BL], f32,
                                                   tag=f"cf{d}", bufs=6)
                                    nc.vector.tensor_tensor(
                                        cf[:], sg[:, SL_F, :], c_prev[d][:],
                                        op=OP.mult)
                                xx = work.tile([128, 2, BL], f32,
                                               tag=f"xx{d}", bufs=6)
                                nc.vector.tensor_tensor(
                                    xx[:], sg[:, SL_I, :], sg[:, SL_G, :],
                                    op=OP.mult)
                                t1 = work.tile([128, 2, BL], f32,
                                               tag=f"t1{d}", bufs=6)
                                nc.vector.scalar_tensor_tensor(
                                    out=t1[:], in0=xx[:], scalar=2.0,
                                    in1=sg[:, SL_I, :], op0=OP.mult,
                                    op1=OP.subtract)
                                if first:
                                    cn = t1
                                else:
                                    cn = work.tile([128, 2, BL], f32,
                                                   tag=f"c{d}", bufs=4)
                                    nc.vector.tensor_tensor(cn[:], t1[:],
                                                            cf[:], op=OP.add)
                                c_prev[d] = cn
                                th = work.tile([128, 2, BL], f32,
                                               tag=f"th{d}", bufs=6)
                                if SPLIT_TAIL:
                                    for kc in range(NKH):
                                        nc.scalar.activation(
                                            th[:, kc, :], cn[:, kc, :],
                                            AF.Tanh)
                                        nc.vector.tensor_tensor(
                                            out_feat[:, d * NKH + kc,
                                                     col:col + BL],
                                            sg[:, 4 + kc, :], th[:, kc, :],
                                            op=OP.mult)
                                else:
                                    nc.scalar.activation(th[:], cn[:],
                                                         AF.Tanh)
                                    nc.vector.tensor_tensor(
                                        out_feat[:, d * NKH:(d + 1) * NKH,
                                                 col:col + BL],
                                        sg[:, SL_O, :], th[:], op=OP.mult)
                            # JIT: one xproj half-unit every other step,
                            # emitted one window ahead of its consumers
                            if jit_units:
                                w = step // SN
                                if w < len(jit_units) and step % 2 == 0:
                                    q = jit_units[w]
                                    if q:
                                        emit_half(*q.pop(0))

                with tc.tile_pool(name="xpool", bufs=1) as xpool:
                    xproj = [xpool.tile([128, NM, TOK], f16, name=f"xproj{d}")
                             for d in range(2)]
                    lstm_layer(0, bufA, bufB, xproj)
                    lstm_layer(1, bufB, bufA, xproj)
                    lstm_layer(2, bufA, bufB, xproj)
                L = bufB     # final lstm output [128, NF, TOK] fp16
                tn = bufA    # reuse as tanh/fused buffer

                # ---------- phase 3: attention + emission feats ----------
                with tc.tile_pool(name="apool", bufs=1) as apool, \
                     tc.tile_pool(name="ps_att", bufs=1, space=PSUM) as ps_att:
                    aw1_t = apool.tile([128, NF, HD], f16)
                    nc.sync.dma_start(
                        aw1_t[:], aw1_d[:].rearrange("(k p) g -> p k g", p=128))
                    ab1_t = apool.tile([128, NF], f32)
                    nc.sync.dma_start(
                        ab1_t[:], ab1_d[:].rearrange("(m p) -> p m", p=128))
                    for m in range(NF):
                        for n in range(NN):
                            ps = ps_att.tile([128, NW], f32, tag="gemm", bufs=3)
                            for k in range(NF):
                                nc.tensor.matmul(
                                    ps[:], lhsT=aw1_t[:, k, m * 128:(m + 1) * 128],
                                    rhs=L[:, k, n * NW:(n + 1) * NW],
                                    start=(k == 0), stop=(k == NF - 1))
                            nc.scalar.activation(tn[:, m, n * NW:(n + 1) * NW],
                                                 ps[:], AF.Tanh,
                                                 bias=ab1_t[:, m:m + 1])
                    aw2_t = apool.tile([128, NF], f16)
                    nc.sync.dma_start(aw2_t[:],
                                      aw2_d[:].rearrange("(k p) -> p k", p=128))
                    en = apool.tile([1, TOK], f32)
                    for n in range(NN):
                        eps = ps_att.tile([1, NW], f32, tag="gemm", bufs=3)
                        for k in range(NF):
                            nc.tensor.matmul(eps[:], lhsT=aw2_t[:, k:k + 1],
                                             rhs=tn[:, k, n * NW:(n + 1) * NW],
                                             start=(k == 0), stop=(k == NF - 1))
                        nc.vector.tensor_copy(en[:, n * NW:(n + 1) * NW], eps[:])
                    # softmax over t for each b; en is [1, (t, b)]
                    env = en[:].rearrange("p (t b) -> p b t", b=BL)
                    mx = apool.tile([1, BL], f32)
                    nc.vector.tensor_reduce(mx[:], env, axis=AX.X, op=OP.max)
                    e2 = apool.tile([1, TOK], f32)
                    nc.vector.tensor_tensor(
                        e2[:].rearrange("p (t b) -> p b t", b=BL), env,
                        mx[:].unsqueeze(2).to_broadcast([1, BL, TT]),
                        op=OP.subtract)
                    ex = apool.tile([1, TOK], f32)
                    nc.scalar.activation(ex[:], e2[:], AF.Exp)
                    sm = apool.tile([1, BL], f32)
                    nc.vector.tensor_reduce(
                        sm[:], ex[:].rearrange("p (t b) -> p b t", b=BL),
                        axis=AX.X, op=OP.add)
                    rc = apool.tile([1, BL], f32)
                    nc.vector.reciprocal(rc[:], sm[:])
                    wp1 = apool.tile([1, TOK], f16)
                    nc.vector.tensor_tensor(
                        wp1[:].rearrange("p (t b) -> p b t", b=BL),
                        ex[:].rearrange("p (t b) -> p b t", b=BL),
                        rc[:].unsqueeze(2).to_broadcast([1, BL, TT]),
                        op=OP.mult)
                    nc.vector.tensor_scalar_add(wp1[:], wp1[:], 1.0)
                    for n in range(NN):
                        wb = ps_att.tile([128, NW], f32, tag="gemm", bufs=3)
                        nc.tensor.matmul(wb[:], lhsT=ones1[:],
                                         rhs=wp1[:, n * NW:(n + 1) * NW],
                                         start=True, stop=True)
                        for fc in range(NF):
                            nc.vector.tensor_tensor(
                                tn[:, fc, n * NW:(n + 1) * NW],
                                L[:, fc, n * NW:(n + 1) * NW], wb[:], op=OP.mult)
                    fu = tn  # fused features now live in bufA
                    hw_t = apool.tile([128, NF, TAGS], f16)
                    nc.sync.dma_start(
                        hw_t[:], hw_d[:].rearrange("(k p) t -> p k t", p=128))
                    hb_t = apool.tile([TAGS, 1], f32)
                    nc.sync.dma_start(hb_t[:],
                                      hb_d[:].rearrange("(t a) -> t a", a=1))
                    feats = pers.tile([TAGS, TOK], f32)
                    for n in range(NN):
                        fps = ps_att.tile([TAGS, NW], f32, tag="gemm", bufs=3)
                        for k in range(NF):
                            nc.tensor.matmul(fps[:], lhsT=hw_t[:, k, :],
                                             rhs=fu[:, k, n * NW:(n + 1) * NW],
                                             start=(k == 0), stop=(k == NF - 1))
                        nc.scalar.add(feats[:, n * NW:(n + 1) * NW], fps[:],
                                      hb_t[:])

                # ---------- phase 4: CRF forward (scaled, prob space) ----------
                # Batch-major on partitions: alpha is [BL, TAGS]; one step is
                # two back-to-back DVE ops (mult by precomputed Ae[b,t,j,i],
                # reduce over i) -- no cross-engine ping-pong on the chain.
                with tc.tile_pool(name="cpool", bufs=1) as cpool:
                  with tc.tile_pool(name="ps_crf", bufs=1, space=PSUM) as ps_crf:
                      tr_t = cpool.tile([TAGS, TAGS], f32)
                      nc.sync.dma_start(tr_t[:], tr_d[:])
                      trT_t = cpool.tile([TAGS, TAGS], f32)
                      nc.sync.dma_start(trT_t[:], trT_d[:])
                      iot6 = cpool.tile([TAGS, 1], f32)
                      nc.vector.tensor_copy(iot6[:], iop[0:TAGS, :])
                      ones6 = cpool.tile([TAGS, 1], f32)
                      nc.vector.memset(ones6[:], 1.0)
                      # flat trans (j,i) on one partition; exp; replicate to
                      # BL partitions with a ones-matmul broadcast
                      a1 = cpool.tile([1, TAGS * TAGS], f32)
                      nc.sync.dma_start(a1[:],
                                        tr_d[:].rearrange("j i -> (j i)"))
                      ea1 = cpool.tile([1, TAGS * TAGS + TAGS], f32)
                      nc.scalar.activation(ea1[:, :TAGS * TAGS], a1[:], AF.Exp)
                      nc.scalar.activation(
                          ea1[:, TAGS * TAGS:],
                          a1[:, STOP * TAGS:(STOP + 1) * TAGS], AF.Exp)
                      ones8f = cpool.tile([1, BL], f32)
                      nc.vector.memset(ones8f[:], 1.0)
                      rep_ps = ps_crf.tile([BL, TAGS * TAGS + TAGS], f32,
                                           tag="rep", bufs=1)
                      nc.tensor.matmul(rep_ps[:], lhsT=ones8f[:], rhs=ea1[:],
                                       start=True, stop=True)
                      a8 = cpool.tile([BL, TAGS * TAGS], f32)
                      nc.vector.tensor_copy(a8[:], rep_ps[:, :TAGS * TAGS])
                      es8 = cpool.tile([BL, TAGS], f32)
                      nc.vector.tensor_copy(es8[:], rep_ps[:, TAGS * TAGS:])
                      # exp(feats) then permute (j,(t,b)) -> (b,(t,j))
                      expF = cpool.tile([TAGS, TOK], f32)
                      nc.scalar.activation(expF[:], feats[:], AF.Exp)
                      expT = cpool.tile([BL, TT * TAGS], f32)
                      # permute (j,(t,b)) -> (b,(t,j)) via a DRAM bounce
                      # (partition-crossing SBUF->SBUF APs don't balance)
                      ef_d = nc.dram_tensor(f"ef_scratch{_rep}", [TAGS, TOK],
                                            f32, kind="Internal")
                      nc.sync.dma_start(ef_d[:], expF[:])
                      expT3 = expT[:].rearrange("b (t j) -> b t j", j=TAGS)
                      for j in range(TAGS):
                          nc.sync.dma_start(
                              expT3[:, :, j:j + 1],
                              ef_d[j:j + 1, :].rearrange(
                                  "a (t b) -> b t a", b=BL))
                      # Ae[b,t,j,i] = expT[b,t,j] * exp(trans)[j,i]
                      ae = cpool.tile([BL, TT, TAGS, TAGS], f32)
                      nc.vector.tensor_tensor(
                          ae[:],
                          expT[:].rearrange("b (t j) -> b t j", j=TAGS)
                          .unsqueeze(3).to_broadcast([BL, TT, TAGS, TAGS]),
                          a8[:].rearrange("b (j i) -> b j i", i=TAGS)
                          .unsqueeze(1).to_broadcast([BL, TT, TAGS, TAGS]),
                          op=OP.mult)
                      # alpha0[b,i] = (i == START)
                      iof8 = cpool.tile([BL, TAGS], f32)
                      nc.vector.tensor_copy(iof8[:], iof[0:BL, 0:TAGS])
                      al0 = cpool.tile([BL, TAGS], f32)
                      nc.vector.tensor_scalar(
                          out=al0[:], in0=iof8[:], scalar1=float(START),
                          scalar2=None, op0=OP.is_equal)
                      logz8 = cpool.tile([BL, 1], f32)
                      nc.vector.memset(logz8[:], 0.0)
                      lz = logz8
                      al = al0
                      for t in range(TT):
                          u = work.tile([BL, TAGS, TAGS], f32, tag="crfu",
                                        bufs=4)
                          nc.vector.tensor_tensor(
                              u[:],
                              al[:].unsqueeze(1)
                              .to_broadcast([BL, TAGS, TAGS]),
                              ae[:, t], op=OP.mult)
                          an = work.tile([BL, TAGS], f32, tag="crfa", bufs=4)
                          nc.vector.tensor_reduce(an[:], u[:], axis=AX.X,
                                                  op=OP.add)
                          al = an
                          if t % RSC == RSC - 1 and t != TT - 1:
                              s = work.tile([BL, 1], f32, tag="crfs", bufs=2)
                              nc.vector.tensor_reduce(s[:], al[:], axis=AX.X,
                                                      op=OP.add)
                              rc = work.tile([BL, 1], f32, tag="crfr", bufs=2)
                              nc.vector.reciprocal(rc[:], s[:])
                              al2 = work.tile([BL, TAGS], f32, tag="crfa",
                                              bufs=4)
                              nc.vector.tensor_tensor(
                                  al2[:], al[:],
                                  rc[:].to_broadcast([BL, TAGS]), op=OP.mult)
                              al = al2
                              lg = work.tile([BL, 1], f32, tag="crflg", bufs=2)
                              nc.scalar.activation(lg[:], s[:], AF.Ln)
                              lzn = work.tile([BL, 1], f32, tag="crflz",
                                              bufs=2)
                              nc.vector.tensor_tensor(lzn[:], lz[:], lg[:],
                                                      op=OP.add)
                              lz = lzn
                      fin = work.tile([BL, TAGS], f32, tag="crfu", bufs=4)
                      nc.vector.tensor_tensor(fin[:], al[:], es8[:],
                                              op=OP.mult)
                      fsum = work.tile([BL, 1], f32, tag="crfs", bufs=2)
                      nc.vector.tensor_reduce(fsum[:], fin[:], axis=AX.X,
                                              op=OP.add)
                      fsl = cpool.tile([BL, 1], f32)
                      nc.scalar.activation(fsl[:], fsum[:], AF.Ln)
                      fsb = cpool.tile([BL, 1], f32)
                      nc.vector.tensor_tensor(fsb[:], fsl[:], lz[:], op=OP.add)
                      fs2_ps = ps_crf.tile([1, BL], f32, tag="fs2", bufs=1)
                      nc.tensor.transpose(fs2_ps[:], fsb[:], ident[0:BL, 0:BL])
                      fs2 = pers.tile([1, BL], f32)
                      nc.vector.tensor_copy(fs2[:], fs2_ps[:])

                  # ---------- phase 5: gold path score ----------
                  with tc.tile_pool(name="ps_gold", bufs=1, space=PSUM) as ps_gold:
                      tags_sb = cpool.tile([1, TOK], i32)
                      nc.sync.dma_start(tags_sb[:],
                                        tags_d[:].rearrange("(a t) -> a t", a=1))
                      tagf = cpool.tile([1, TOK], f16)
                      nc.vector.tensor_copy(tagf[:], tags_sb[:])
                      iotf = iot6
                      oh = cpool.tile([TAGS, TOK], f32)
                      for n in range(NN):
                          tb_ps = ps_gold.tile([TAGS, NW], f32, tag="gemm", bufs=2)
                          nc.tensor.matmul(tb_ps[:], lhsT=ones1[:, 0:TAGS],
                                           rhs=tagf[:, n * NW:(n + 1) * NW],
                                           start=True, stop=True)
                          nc.vector.tensor_tensor(
                              oh[:, n * NW:(n + 1) * NW], tb_ps[:],
                              iotf[:].to_broadcast([TAGS, NW]), op=OP.is_equal)
                      tcol = cpool.tile([TAGS, TOK], f32)
                      for n in range(NN):
                          tc_ps = ps_gold.tile([TAGS, NW], f32, tag="gemm", bufs=2)
                          nc.tensor.matmul(tc_ps[:], lhsT=tr_t[:],
                                           rhs=oh[:, n * NW:(n + 1) * NW],
                                           start=True, stop=True)
                          nc.vector.tensor_copy(tcol[:, n * NW:(n + 1) * NW],
                                                tc_ps[:])
                      gg = cpool.tile([TAGS, TOK], f32)
                      nc.vector.tensor_tensor(gg[:], oh[:], feats[:], op=OP.mult)
                      g2 = cpool.tile([TAGS, TOK], f32)
                      nc.vector.tensor_tensor(g2[:, BL:], oh[:, 0:TOK - BL],
                                              tcol[:, BL:], op=OP.mult)
                      nc.vector.tensor_tensor(
                          g2[:, 0:BL], oh[:, 0:BL],
                          tr_t[:, START:START + 1].to_broadcast([TAGS, BL]),
                          op=OP.mult)
                      gl = cpool.tile([TAGS, BL], f32)
                      nc.vector.tensor_tensor(
                          gl[:], oh[:, TOK - BL:],
                          trT_t[:, STOP:STOP + 1].to_broadcast([TAGS, BL]),
                          op=OP.mult)
                      gold_ps = ps_gold.tile([1, TOK], f32, tag="gold", bufs=1)
                      for n in range(NN):
                          nsl = slice(n * NW, (n + 1) * NW)
                          nc.tensor.matmul(gold_ps[:, nsl], lhsT=ones6[:],
                                           rhs=gg[:, nsl], start=True, stop=False)
                          last = (n == NN - 1)
                          nc.tensor.matmul(gold_ps[:, nsl], lhsT=ones6[:],
                                           rhs=g2[:, nsl], start=False,
                                           stop=not last)
                      nc.tensor.matmul(gold_ps[:, TOK - BL:], lhsT=ones6[:],
                                       rhs=gl[:], start=False, stop=True)
                      gold = pers.tile([1, BL], f32)
                      nc.vector.tensor_reduce(
                          gold[:], gold_ps[:].rearrange("p (t b) -> p b t", b=BL),
                          axis=AX.X, op=OP.add)
                      loss = pers.tile([1, BL], f32)
                      nc.vector.tensor_tensor(loss[:], fs2[:], gold[:],
                                              op=OP.subtract)
                      nc.sync.dma_start(out_d[:], loss[:])

            for _rep in range(reps):
                if _rep:
                    tc.strict_bb_all_engine_barrier()
                emit_pipeline()

    if legalize:
        _legalize_multi_waits(nc)
    return nc


_LW_COUNT = [0]


_SELF_SEM = {mybir.EngineType.DVE: "DVE_", mybir.EngineType.Activation: "Activation_"}


def _legalize_multi_waits(nc):
    """This container's walrus accepts at most ONE sync wait per TPB
    instruction ("Too many sync wait commands" in codegen otherwise), while
    Tile freely attaches several.  Split: keep one wait on the instruction
    and hoist the rest onto standalone InstEventSemaphore instructions on
    the same engine immediately before it (engine-order preserved, so the
    semantics are identical; the stall just happens one slot earlier)."""
    n_new = 0
    for fn in nc.m.functions:
        for bb in fn.blocks:
            out = []
            changed = False
            for inst in bb.instructions:
                si = inst.sync_info
                waits = list(si.on_wait) if si is not None else []
                pref = _SELF_SEM.get(inst.engine)
                if pref is not None and len(waits) > 1:
                    # DVE/ACT are strict in-order single pipelines: a wait on
                    # the engine's own completion semaphore is subsumed by
                    # program order. Drop it (only when other waits remain).
                    kept = [w for w in waits
                            if not str(w.ant_name).startswith(pref)]
                    if kept:
                        waits = kept
                        inst.sync_info = mybir.SyncInfo(
                            on_wait=waits, on_update=list(si.on_update))
                        si = inst.sync_info
                        changed = True
                cap = 1
                if len(waits) > cap:
                    for w in waits[:-cap]:
                        _LW_COUNT[0] += 1
                        es = mybir.InstEventSemaphore(
                            name=f"I-lw{_LW_COUNT[0]}", ins=[], outs=[])
                        es.engine = inst.engine
                        es.sync_info = mybir.SyncInfo(on_wait=[w],
                                                      on_update=[])
                        out.append(es)
                        n_new += 1
                    inst.sync_info = mybir.SyncInfo(
                        on_wait=waits[-cap:], on_update=list(si.on_update))
                    changed = True
                out.append(inst)
            if changed:
                bb.instructions = out
    return n_new


_PROG_CACHE = {}


def _get_program(t_steps=T):
    if t_steps not in _PROG_CACHE:
        _PROG_CACHE[t_steps] = build_program(t_steps)
    return _PROG_CACHE[t_steps]


def _slot_reorder_cols(wT):
    """Reorder the 4H (=1024) columns of [in, 4H] into slot order, and
    pre-scale the g-gate slots by 2 (tanh(g) = 2*sigmoid(2g) - 1, so the
    kernel runs a single Sigmoid over all gates)."""
    chunks = [wT[:, c * 128:(c + 1) * 128].copy() for c in SLOT_SRC]
    chunks[6] = chunks[6] * 2.0
    chunks[7] = chunks[7] * 2.0
    return np.ascontiguousarray(np.concatenate(chunks, axis=1))


def _slot_reorder_vec(v):
    chunks = [v[c * 128:(c + 1) * 128].copy() for c in SLOT_SRC]
    chunks[6] = chunks[6] * 2.0
    chunks[7] = chunks[7] * 2.0
    return np.ascontiguousarray(np.concatenate(chunks))


def prep_inputs(sentences, tags, mask, embed, w_ih, w_hh, b_ih, b_hh,
                aw1, ab1, aw2, ab2, hw, hb, transitions, t_steps=T):
    sent = np.asarray(sentences).astype(np.int32)
    tg = np.asarray(tags).astype(np.int32)
    emb = np.ascontiguousarray(np.asarray(embed, dtype=np.float32))
    w_ih = np.asarray(w_ih, dtype=np.float32)
    w_hh = np.asarray(w_hh, dtype=np.float32)
    b_ih = np.asarray(b_ih, dtype=np.float32)
    b_hh = np.asarray(b_hh, dtype=np.float32)
    trans = np.clip(np.asarray(transitions, dtype=np.float32), -TCLIP, TCLIP)
    wh_np = ml_dtypes.float8_e4m3 if FP8_WHH else np.float16

    shared = {"embed": emb,
              "ident8": np.ascontiguousarray(np.eye(128).astype(wh_np)),
              "aw1T": np.ascontiguousarray(
                  np.asarray(aw1, np.float32).T.astype(np.float16)),
              "ab1": np.ascontiguousarray(np.asarray(ab1, np.float32)),
              "aw2": np.ascontiguousarray(
                  np.asarray(aw2, np.float32).astype(np.float16)),
              "hwT": np.ascontiguousarray(
                  np.asarray(hw, np.float32).T.astype(np.float16)),
              "hb": np.ascontiguousarray(np.asarray(hb, np.float32)),
              "trans": np.ascontiguousarray(trans),
              "transT": np.ascontiguousarray(trans.T)}
    for l in range(3):
        for d in range(2):
            shared[f"wihT_{l}_{d}"] = _slot_reorder_cols(
                w_ih[l, d].T).astype(np.float16)
            shared[f"whhT_{l}_{d}"] = _slot_reorder_cols(
                w_hh[l, d].T).astype(wh_np)
            shared[f"bias_{l}_{d}"] = _slot_reorder_vec(
                b_ih[l, d] + b_hh[l, d])

    in_maps = []
    for c in range(NCORES):
        sh = slice(c * BL, (c + 1) * BL)
        m = dict(shared)
        # (t, b) order, b innermost
        m["tok_ids"] = np.ascontiguousarray(
            sent[sh, :t_steps].T).reshape(-1)
        m["tags_tb"] = np.ascontiguousarray(tg[sh, :t_steps].T).reshape(-1)
        in_maps.append(m)
    return in_maps


def kernel(sentences, tags, mask, embed, w_ih, w_hh, b_ih, b_hh,
           aw1, ab1, aw2, ab2, hw, hb, transitions, _trace=False):
    nc = _get_program()
    in_maps = prep_inputs(sentences, tags, mask, embed, w_ih, w_hh,
                          b_ih, b_hh, aw1, ab1, aw2, ab2, hw, hb,
                          transitions)
    res = run_bass_kernel_spmd(nc, in_maps, core_ids=list(range(NCORES)),
                               trace=_trace)
    parts = np.concatenate([r["loss_part"].reshape(-1) for r in res.results])
    kernel.last_result = res
    return np.float32(parts.mean())


# revision 14
# speedup vs baseline: 1.9854x; 1.9785x over previous
"""BiLSTM-CRF loss kernel for 8 Trainium2 NeuronCores.

Sharding: data-parallel over batch (64 -> 8 per core). Each core runs the
full 3-layer BiLSTM + attention + CRF on its shard and returns the
per-sample (forward_score - gold_score); the host averages them.

On-chip layout: all sequence features are kept transposed as
[feature_dim(partitions), time*batch(free)] so that LSTM gates land on 128
partitions.  Input projections (Wih @ x for every timestep) are computed
as fp16 GEMMs interleaved just-in-time into the recurrence's PE idle gaps
(window-ahead emission).  The sequential recurrence per step is Whh @ h
with Whh stored fp8(e4m3) -- LDWEIGHTS is the dominant PE cost at free
dim BL=8 and fp8 weight loads are ~2x faster (FWL); h stays fp16.
The per-step tail is split by h-chunk (k-major matmuls + split tanh/mult)
so the next step's k0 matmuls start as soon as chunk0 of h is ready.
The CRF forward pass runs in probability space (scaled forward algorithm).
"""

import os
import sys

import numpy as np
import ml_dtypes

sys.path.insert(0, "/opt/trn_rl_repo")

import concourse.bass as bass  # noqa: E402
import concourse.mybir as mybir  # noqa: E402
from concourse import tile  # noqa: E402
from concourse.bass_utils import run_bass_kernel_spmd  # noqa: E402

# ---- problem constants (hardcoded per harness contract) ----
V, E, H, HD, G = 8000, 512, 256, 512, 1024
TAGS, START, STOP = 6, 4, 5
B, T = 64, 256
NCORES = 8
BL = B // NCORES          # local batch = 8
NKE = E // 128            # k-chunks over input features (4)
NKH = H // 128            # k-chunks over hidden (2)
NM = G // 128             # gate chunks (8)
NF = HD // 128            # feature chunks (4)
RSC = 8                   # CRF renormalization interval
TCLIP = 30000.0           # replaces -1e6 in transitions (exp() still == 0)

# dev flags (defaults are the shipped config; env only for experiments)
FP8_WHH = os.environ.get("K_FP8", "1") == "1"
SPLIT_TAIL = os.environ.get("K_SPLIT_TAIL", "1") == "1"
XJIT = os.environ.get("K_XJIT", "1") == "1"

f32 = mybir.dt.float32
f16 = mybir.dt.float16
f8 = mybir.dt.float8e4
i32 = mybir.dt.int32
AF = mybir.ActivationFunctionType
OP = mybir.AluOpType
AX = mybir.AxisListType
PSUM = bass.MemorySpace.PSUM

# gate slot order in PSUM/gates tiles: [i0,i1,f0,f1,o0,o1,g0,g1]
# natural (PyTorch) chunk order in the 4H dim is i(0,1) f(2,3) g(4,5) o(6,7)
SLOT_SRC = [0, 1, 2, 3, 6, 7, 4, 5]
SL_I, SL_F, SL_O, SL_G = slice(0, 2), slice(2, 4), slice(4, 6), slice(6, 8)

WH_DT = f8 if FP8_WHH else f16


def build_program(t_steps=T, reps=1, legalize=True, ablate=None):
    """Build the (single, SPMD-identical) Bass program. Returns nc.

    ablate (dev-only, breaks correctness; timing attribution):
      'norec'    -- skip the recurrence pointwise+mm per-step body
      'reconly'  -- only the recurrence (skip embed/xproj/attn/crf/gold)
      'nommh'    -- recurrence without the 16 Whh matmuls per dir-step
      'nopoint'  -- recurrence without sigmoid/DVE/tanh pointwise chain
    """
    TT = t_steps
    TOK = TT * BL
    NGl = (TOK + 127) // 128
    NW = 512 if TOK >= 512 else TOK   # gemm n-tile width
    NN = TOK // NW
    SN = TT // NN                     # recurrence steps per xproj n-tile

    nc = bass.Bass()

    # ---- DRAM I/O ----
    tok_d = nc.dram_tensor("tok_ids", [TOK], i32, kind="ExternalInput")
    tags_d = nc.dram_tensor("tags_tb", [TOK], i32, kind="ExternalInput")
    emb_d = nc.dram_tensor("embed", [V, E], f32, kind="ExternalInput")
    wih_d, whh_d, bias_d = {}, {}, {}
    for l in range(3):
        for d in range(2):
            wih_d[l, d] = nc.dram_tensor(f"wihT_{l}_{d}", [E, G], f16,
                                         kind="ExternalInput")
            whh_d[l, d] = nc.dram_tensor(f"whhT_{l}_{d}", [H, G], WH_DT,
                                         kind="ExternalInput")
            bias_d[l, d] = nc.dram_tensor(f"bias_{l}_{d}", [G], f32,
                                          kind="ExternalInput")
    id8_d = nc.dram_tensor("ident8", [128, 128], WH_DT, kind="ExternalInput")
    aw1_d = nc.dram_tensor("aw1T", [HD, HD], f16, kind="ExternalInput")
    ab1_d = nc.dram_tensor("ab1", [HD], f32, kind="ExternalInput")
    aw2_d = nc.dram_tensor("aw2", [HD], f16, kind="ExternalInput")
    hw_d = nc.dram_tensor("hwT", [HD, TAGS], f16, kind="ExternalInput")
    hb_d = nc.dram_tensor("hb", [TAGS], f32, kind="ExternalInput")
    tr_d = nc.dram_tensor("trans", [TAGS, TAGS], f32, kind="ExternalInput")
    trT_d = nc.dram_tensor("transT", [TAGS, TAGS], f32, kind="ExternalInput")
    out_d = nc.dram_tensor("loss_part", [1, BL], f32, kind="ExternalOutput")

    with tile.TileContext(nc) as tc:
        with tc.tile_pool(name="pers", bufs=1) as pers, \
             tc.tile_pool(name="work", bufs=4) as work:

            # ---------- constants / indices ----------
            idx = pers.tile([128, NGl], i32)
            nc.sync.dma_start(idx[:],
                              tok_d[:].rearrange("(g p) -> p g", p=128))
            iop = pers.tile([128, 1], i32)
            nc.gpsimd.iota(iop[:], pattern=[[0, 1]], base=0,
                           channel_multiplier=1)
            iof = pers.tile([128, 128], i32)
            nc.gpsimd.iota(iof[:], pattern=[[1, 128]], base=0,
                           channel_multiplier=0)
            ident = pers.tile([128, 128], f32)
            nc.vector.tensor_tensor(ident[:], iop[:].to_broadcast([128, 128]),
                                    iof[:], op=OP.is_equal)
            ident8 = pers.tile([128, 128], WH_DT)
            nc.sync.dma_start(ident8[:], id8_d[:])
            ones1 = pers.tile([1, 128], f16)
            nc.vector.memset(ones1[:], 1.0)

            # big feature buffers, [128, chunk, tok] fp16
            bufA = pers.tile([128, NF, TOK], f16)
            bufB = pers.tile([128, NF, TOK], f16)

            def emit_pipeline():
                if ablate in ('norec', 'nopoint', 'reconly'):
                    # 1-element writes so Tile allocates the never-written
                    # buffers that ablated phases would have produced
                    nc.vector.memset(bufB[:, 0:1, 0:1], 0.0)
                    if ablate == 'reconly':
                        nc.vector.memset(bufA[:, 0:1, 0:1], 0.0)
                # ---------- phase 1: embedding gather + transpose ----------
                if ablate != 'reconly':
                  with tc.tile_pool(name="ps_emb", bufs=1, space=PSUM) as ps_emb:
                    for g in range(NGl):
                        gt = work.tile([128, E], f32, tag="gt", bufs=3)
                        nc.gpsimd.indirect_dma_start(
                            out=gt[:], out_offset=None, in_=emb_d[:],
                            in_offset=bass.IndirectOffsetOnAxis(
                                ap=idx[:, g:g + 1], axis=0))
                        for e in range(NKE):
                            tp = ps_emb.tile([128, 128], f32, tag="tp", bufs=4)
                            nc.tensor.transpose(
                                tp[:], gt[:, e * 128:(e + 1) * 128], ident[:])
                            nc.vector.tensor_copy(
                                bufA[:, e, g * 128:(g + 1) * 128], tp[:])

                # ---------- phase 2: LSTM layers ----------
                def lstm_layer(l, in_feat, out_feat, xproj):
                    with tc.tile_pool(name=f"ps_l{l}", bufs=1,
                                      space=PSUM) as psl:
                        # --- weight/bias DMA ---
                        wih_t, whh_t, bias_t = {}, {}, {}
                        for d in range(2):
                            wih_t[d] = work.tile([128, NKE, G], f16,
                                                 name=f"wih_t{d}",
                                                 tag=f"wih{d}", bufs=2)
                            nc.sync.dma_start(
                                wih_t[d][:],
                                wih_d[l, d][:].rearrange("(k p) g -> p k g",
                                                         p=128))
                            whh_t[d] = work.tile([128, NKH, G], WH_DT,
                                                 name=f"whh_t{d}",
                                                 tag=f"whh{d}", bufs=2)
                            nc.sync.dma_start(
                                whh_t[d][:],
                                whh_d[l, d][:].rearrange("(k p) g -> p k g",
                                                         p=128))
                            bias_t[d] = work.tile([128, NM], f32,
                                                  name=f"bias_t{d}",
                                                  tag=f"bias{d}", bufs=2)
                            nc.sync.dma_start(
                                bias_t[d][:],
                                bias_d[l, d][:].rearrange("(m p) -> p m",
                                                          p=128))

                        # --- xproj unit emitters (one unit = d, m, n-tile;
                        #     emitted as two k-halves so each PE burst is
                        #     short enough to hide in a recurrence gap) ---
                        xpps_live = {}

                        def emit_half(d, m, n, half):
                            if half == 0:
                                ps = psl.tile([128, NW], f32, name="xpps",
                                              tag="xpps", bufs=2)
                                xpps_live[(d, m, n)] = ps
                            else:
                                ps = xpps_live.pop((d, m, n))
                            for k in (2 * half, 2 * half + 1):
                                nc.tensor.matmul(
                                    ps[:],
                                    lhsT=wih_t[d][:, k, m * 128:(m + 1) * 128],
                                    rhs=in_feat[:, k, n * NW:(n + 1) * NW],
                                    start=(k == 0), stop=(k == NKE - 1))
                            if half == 1:
                                nc.scalar.add(
                                    xproj[d][:, m, n * NW:(n + 1) * NW],
                                    ps[:], bias_t[d][:, m:m + 1])

                        # upfront: first-needed n-tiles for both dirs
                        jit_units = []   # remaining, window-ordered
                        if ablate == 'reconly':
                            pass
                        elif XJIT and NN > 1:
                            for d, n in ((0, 0), (1, NN - 1)):
                                for m in range(NM):
                                    emit_half(d, m, n, 0)
                                    emit_half(d, m, n, 1)
                            for w in range(NN - 1):
                                wu = []
                                for m in range(NM):
                                    wu.append((0, m, w + 1))
                                    wu.append((1, m, NN - 2 - w))
                                halves = []
                                for u in wu:
                                    halves.append(u + (0,))
                                    halves.append(u + (1,))
                                jit_units.append(halves)
                        else:
                            for d in range(2):
                                for n in range(NN):
                                    for m in range(NM):
                                        emit_half(d, m, n, 0)
                                        emit_half(d, m, n, 1)

                        # --- recurrence ---
                        c_prev = {0: None, 1: None}
                        for step in range(TT):
                            first = step == 0
                            cols = [step * BL, (TT - 1 - step) * BL]
                            for d in range(2):
                                if ablate == 'norec':
                                    continue
                                col = cols[d]
                                gp = psl.tile([128, NM, BL], f32,
                                              name=f"gp{d}", tag=f"gp{d}",
                                              bufs=3)
                                nc.tensor.matmul(
                                    gp[:], lhsT=ident8[:],
                                    rhs=xproj[d][:, :, col:col + BL],
                                    start=True, stop=first or ablate == 'nommh')
                                if not first and ablate != 'nommh':
                                    h_col = col + (BL if d else -BL)
                                    for k in range(NKH):
                                        for m in range(NM):
                                            nc.tensor.matmul(
                                                gp[:, m, :],
                                                lhsT=whh_t[d][
                                                    :, k,
                                                    m * 128:(m + 1) * 128],
                                                rhs=out_feat[:, d * NKH + k,
                                                             h_col:h_col + BL],
                                                start=False,
                                                stop=(k == NKH - 1
                                                      and m == NM - 1))
                                if ablate == 'nopoint':
                                    continue
                                sg = work.tile([128, 8, BL], f32,
                                               tag=f"sg{d}", bufs=6)
                                nc.scalar.activation(sg[:], gp[:], AF.Sigmoid)
                                if not first:
                                    cf = work.tile([128, 2, BL], f32,
                                                   tag=f"cf{d}", bufs=6)
                                    nc.vector.tensor_tensor(
                                        cf[:], sg[:, SL_F, :], c_prev[d][:],
                                        op=OP.mult)
                                xx = work.tile([128, 2, BL], f32,
                                               tag=f"xx{d}", bufs=6)
                                nc.vector.tensor_tensor(
                                    xx[:], sg[:, SL_I, :], sg[:, SL_G, :],
                                    op=OP.mult)
                                t1 = work.tile([128, 2, BL], f32,
                                               tag=f"t1{d}", bufs=6)
                                nc.vector.scalar_tensor_tensor(
                                    out=t1[:], in0=xx[:], scalar=2.0,
                                    in1=sg[:, SL_I, :], op0=OP.mult,
                                    op1=OP.subtract)
                                if first:
                                    cn = t1
                                else:
                                    cn = work.tile([128, 2, BL], f32,
                                                   tag=f"c{d}", bufs=4)
                                    nc.vector.tensor_tensor(cn[:], t1[:],
                                                            cf[:], op=OP.add)
                                c_prev[d] = cn
                                th = work.tile([128, 2, BL], f32,
                                               tag=f"th{d}", bufs=6)
                                if SPLIT_TAIL:
                                    for kc in range(NKH):
                                        nc.scalar.activation(
                                            th[:, kc, :], cn[:, kc, :],
                                            AF.Tanh)
                                        nc.vector.tensor_tensor(
                                            out_feat[:, d * NKH + kc,
                                                     col:col + BL],
                                            sg[:, 4 + kc, :], th[:, kc, :],
                                            op=OP.mult)
                                else:
                                    nc.scalar.activation(th[:], cn[:],
                                                         AF.Tanh)
                                    nc.vector.tensor_tensor(
                                        out_feat[:, d * NKH:(d + 1) * NKH,
                                                 col:col + BL],
                                        sg[:, SL_O, :], th[:], op=OP.mult)
                            # JIT: one xproj half-unit every other step,
                            # emitted one window ahead of its consumers
                            if jit_units:
                                w = step // SN
                                if w < len(jit_units) and step % 2 == 0:
                                    q = jit_units[w]
                                    if q:
                                        emit_half(*q.pop(0))

                with tc.tile_pool(name="xpool", bufs=1) as xpool:
                    xproj = [xpool.tile([128, NM, TOK], f16, name=f"xproj{d}")
                             for d in range(2)]
                    if ablate == 'reconly':
                        for d in range(2):
                            nc.vector.memset(xproj[d][:, 0:1, 0:1], 0.0)
                    lstm_layer(0, bufA, bufB, xproj)
                    lstm_layer(1, bufB, bufA, xproj)
                    lstm_layer(2, bufA, bufB, xproj)
                if ablate == 'reconly':
                    loss0 = pers.tile([1, BL], f32, name="loss0")
                    nc.vector.memset(loss0[:], 0.0)
                    nc.sync.dma_start(out_d[:], loss0[:])
                    return
                L = bufB     # final lstm output [128, NF, TOK] fp16
                tn = bufA    # reuse as tanh/fused buffer

                # ---------- phase 3: attention + emission feats ----------
                with tc.tile_pool(name="apool", bufs=1) as apool, \
                     tc.tile_pool(name="ps_att", bufs=1, space=PSUM) as ps_att:
                    aw1_t = apool.tile([128, NF, HD], f16)
                    nc.sync.dma_start(
                        aw1_t[:], aw1_d[:].rearrange("(k p) g -> p k g", p=128))
                    ab1_t = apool.tile([128, NF], f32)
                    nc.sync.dma_start(
                        ab1_t[:], ab1_d[:].rearrange("(m p) -> p m", p=128))
                    for m in range(NF):
                        for n in range(NN):
                            ps = ps_att.tile([128, NW], f32, tag="gemm", bufs=3)
                            for k in range(NF):
                                nc.tensor.matmul(
                                    ps[:], lhsT=aw1_t[:, k, m * 128:(m + 1) * 128],
                                    rhs=L[:, k, n * NW:(n + 1) * NW],
                                    start=(k == 0), stop=(k == NF - 1))
                            nc.scalar.activation(tn[:, m, n * NW:(n + 1) * NW],
                                                 ps[:], AF.Tanh,
                                                 bias=ab1_t[:, m:m + 1])
                    aw2_t = apool.tile([128, NF], f16)
                    nc.sync.dma_start(aw2_t[:],
                                      aw2_d[:].rearrange("(k p) -> p k", p=128))
                    en = apool.tile([1, TOK], f32)
                    for n in range(NN):
                        eps = ps_att.tile([1, NW], f32, tag="gemm", bufs=3)
                        for k in range(NF):
                            nc.tensor.matmul(eps[:], lhsT=aw2_t[:, k:k + 1],
                                             rhs=tn[:, k, n * NW:(n + 1) * NW],
                                             start=(k == 0), stop=(k == NF - 1))
                        nc.vector.tensor_copy(en[:, n * NW:(n + 1) * NW], eps[:])
                    # softmax over t for each b; en is [1, (t, b)]
                    env = en[:].rearrange("p (t b) -> p b t", b=BL)
                    mx = apool.tile([1, BL], f32)
                    nc.vector.tensor_reduce(mx[:], env, axis=AX.X, op=OP.max)
                    e2 = apool.tile([1, TOK], f32)
                    nc.vector.tensor_tensor(
                        e2[:].rearrange("p (t b) -> p b t", b=BL), env,
                        mx[:].unsqueeze(2).to_broadcast([1, BL, TT]),
                        op=OP.subtract)
                    ex = apool.tile([1, TOK], f32)
                    nc.scalar.activation(ex[:], e2[:], AF.Exp)
                    sm = apool.tile([1, BL], f32)
                    nc.vector.tensor_reduce(
                        sm[:], ex[:].rearrange("p (t b) -> p b t", b=BL),
                        axis=AX.X, op=OP.add)
                    rc = apool.tile([1, BL], f32)
                    nc.vector.reciprocal(rc[:], sm[:])
                    wp1 = apool.tile([1, TOK], f16)
                    nc.vector.tensor_tensor(
                        wp1[:].rearrange("p (t b) -> p b t", b=BL),
                        ex[:].rearrange("p (t b) -> p b t", b=BL),
                        rc[:].unsqueeze(2).to_broadcast([1, BL, TT]),
                        op=OP.mult)
                    nc.vector.tensor_scalar_add(wp1[:], wp1[:], 1.0)
                    for n in range(NN):
                        wb = ps_att.tile([128, NW], f32, tag="gemm", bufs=3)
                        nc.tensor.matmul(wb[:], lhsT=ones1[:],
                                         rhs=wp1[:, n * NW:(n + 1) * NW],
                                         start=True, stop=True)
                        for fc in range(NF):
                            nc.vector.tensor_tensor(
                                tn[:, fc, n * NW:(n + 1) * NW],
                                L[:, fc, n * NW:(n + 1) * NW], wb[:], op=OP.mult)
                    fu = tn  # fused features now live in bufA
                    hw_t = apool.tile([128, NF, TAGS], f16)
                    nc.sync.dma_start(
                        hw_t[:], hw_d[:].rearrange("(k p) t -> p k t", p=128))
                    hb_t = apool.tile([TAGS, 1], f32)
                    nc.sync.dma_start(hb_t[:],
                                      hb_d[:].rearrange("(t a) -> t a", a=1))
                    feats = pers.tile([TAGS, TOK], f32)
                    for n in range(NN):
                        fps = ps_att.tile([TAGS, NW], f32, tag="gemm", bufs=3)
                        for k in range(NF):
                            nc.tensor.matmul(fps[:], lhsT=hw_t[:, k, :],
                                             rhs=fu[:, k, n * NW:(n + 1) * NW],
                                             start=(k == 0), stop=(k == NF - 1))
                        nc.scalar.add(feats[:, n * NW:(n + 1) * NW], fps[:],
                                      hb_t[:])

                # ---------- phase 4: CRF forward (scaled, prob space) ----------
                # Batch-major on partitions: alpha is [BL, TAGS]; one step is
                # two back-to-back DVE ops (mult by precomputed Ae[b,t,j,i],
                # reduce over i) -- no cross-engine ping-pong on the chain.
                with tc.tile_pool(name="cpool", bufs=1) as cpool:
                  with tc.tile_pool(name="ps_crf", bufs=1, space=PSUM) as ps_crf:
                      tr_t = cpool.tile([TAGS, TAGS], f32)
                      nc.sync.dma_start(tr_t[:], tr_d[:])
                      trT_t = cpool.tile([TAGS, TAGS], f32)
                      nc.sync.dma_start(trT_t[:], trT_d[:])
                      iot6 = cpool.tile([TAGS, 1], f32)
                      nc.vector.tensor_copy(iot6[:], iop[0:TAGS, :])
                      ones6 = cpool.tile([TAGS, 1], f32)
                      nc.vector.memset(ones6[:], 1.0)
                      # flat trans (j,i) on one partition; exp; replicate to
                      # BL partitions with a ones-matmul broadcast
                      a1 = cpool.tile([1, TAGS * TAGS], f32)
                      nc.sync.dma_start(a1[:],
                                        tr_d[:].rearrange("j i -> (j i)"))
                      ea1 = cpool.tile([1, TAGS * TAGS + TAGS], f32)
                      nc.scalar.activation(ea1[:, :TAGS * TAGS], a1[:], AF.Exp)
                      nc.scalar.activation(
                          ea1[:, TAGS * TAGS:],
                          a1[:, STOP * TAGS:(STOP + 1) * TAGS], AF.Exp)
                      ones8f = cpool.tile([1, BL], f32)
                      nc.vector.memset(ones8f[:], 1.0)
                      rep_ps = ps_crf.tile([BL, TAGS * TAGS + TAGS], f32,
                                           tag="rep", bufs=1)
                      nc.tensor.matmul(rep_ps[:], lhsT=ones8f[:], rhs=ea1[:],
                                       start=True, stop=True)
                      a8 = cpool.tile([BL, TAGS * TAGS], f32)
                      nc.vector.tensor_copy(a8[:], rep_ps[:, :TAGS * TAGS])
                      es8 = cpool.tile([BL, TAGS], f32)
                      nc.vector.tensor_copy(es8[:], rep_ps[:, TAGS * TAGS:])
                      # exp(feats) then permute (j,(t,b)) -> (b,(t,j))
                      expF = cpool.tile([TAGS, TOK], f32)
                      nc.scalar.activation(expF[:], feats[:], AF.Exp)
                      expT = cpool.tile([BL, TT * TAGS], f32)
                      # permute (j,(t,b)) -> (b,(t,j)) via a DRAM bounce
                      # (partition-crossing SBUF->SBUF APs don't balance)
                      ef_d = nc.dram_tensor(f"ef_scratch{_rep}", [TAGS, TOK],
                                            f32, kind="Internal")
                      nc.sync.dma_start(ef_d[:], expF[:])
                      expT3 = expT[:].rearrange("b (t j) -> b t j", j=TAGS)
                      for j in range(TAGS):
                          nc.sync.dma_start(
                              expT3[:, :, j:j + 1],
                              ef_d[j:j + 1, :].rearrange(
                                  "a (t b) -> b t a", b=BL))
                      # Ae[b,t,j,i] = expT[b,t,j] * exp(trans)[j,i]
                      ae = cpool.tile([BL, TT, TAGS, TAGS], f32)
                      nc.vector.tensor_tensor(
                          ae[:],
                          expT[:].rearrange("b (t j) -> b t j", j=TAGS)
                          .unsqueeze(3).to_broadcast([BL, TT, TAGS, TAGS]),
                          a8[:].rearrange("b (j i) -> b j i", i=TAGS)
                          .unsqueeze(1).to_broadcast([BL, TT, TAGS, TAGS]),
                          op=OP.mult)
                      # alpha0[b,i] = (i == START)
                      iof8 = cpool.tile([BL, TAGS], f32)
                      nc.vector.tensor_copy(iof8[:], iof[0:BL, 0:TAGS])
                      al0 = cpool.tile([BL, TAGS], f32)
                      nc.vector.tensor_scalar(
                          out=al0[:], in0=iof8[:], scalar1=float(START),
                          scalar2=None, op0=OP.is_equal)
                      logz8 = cpool.tile([BL, 1], f32)
                      nc.vector.memset(logz8[:], 0.0)
                      lz = logz8
                      al = al0
                      for t in range(TT):
                          u = work.tile([BL, TAGS, TAGS], f32, tag="crfu",
                                        bufs=4)
                          nc.vector.tensor_tensor(
                              u[:],
                              al[:].unsqueeze(1)
                              .to_broadcast([BL, TAGS, TAGS]),
                              ae[:, t], op=OP.mult)
                          an = work.tile([BL, TAGS], f32, tag="crfa", bufs=4)
                          nc.vector.tensor_reduce(an[:], u[:], axis=AX.X,
                                                  op=OP.add)
                          al = an
                          if t % RSC == RSC - 1 and t != TT - 1:
                              s = work.tile([BL, 1], f32, tag="crfs", bufs=2)
                              nc.vector.tensor_reduce(s[:], al[:], axis=AX.X,
                                                      op=OP.add)
                              rc = work.tile([BL, 1], f32, tag="crfr", bufs=2)
                              nc.vector.reciprocal(rc[:], s[:])
                              al2 = work.tile([BL, TAGS], f32, tag="crfa",
                                              bufs=4)
                              nc.vector.tensor_tensor(
                                  al2[:], al[:],
                                  rc[:].to_broadcast([BL, TAGS]), op=OP.mult)
                              al = al2
                              lg = work.tile([BL, 1], f32, tag="crflg", bufs=2)
                              nc.scalar.activation(lg[:], s[:], AF.Ln)
                              lzn = work.tile([BL, 1], f32, tag="crflz",
                                              bufs=2)
                              nc.vector.tensor_tensor(lzn[:], lz[:], lg[:],
                                                      op=OP.add)
                              lz = lzn
                      fin = work.tile([BL, TAGS], f32, tag="crfu", bufs=4)
                      nc.vector.tensor_tensor(fin[:], al[:], es8[:],
                                              op=OP.mult)
                      fsum = work.tile([BL, 1], f32, tag="crfs", bufs=2)
                      nc.vector.tensor_reduce(fsum[:], fin[:], axis=AX.X,
                                              op=OP.add)
                      fsl = cpool.tile([BL, 1], f32)
                      nc.scalar.activation(fsl[:], fsum[:], AF.Ln)
                      fsb = cpool.tile([BL, 1], f32)
                      nc.vector.tensor_tensor(fsb[:], fsl[:], lz[:], op=OP.add)
                      fs2_ps = ps_crf.tile([1, BL], f32, tag="fs2", bufs=1)
                      nc.tensor.transpose(fs2_ps[:], fsb[:], ident[0:BL, 0:BL])
                      fs2 = pers.tile([1, BL], f32)
                      nc.vector.tensor_copy(fs2[:], fs2_ps[:])

                  # ---------- phase 5: gold path score ----------
                  with tc.tile_pool(name="ps_gold", bufs=1, space=PSUM) as ps_gold:
                      tags_sb = cpool.tile([1, TOK], i32)
                      nc.sync.dma_start(tags_sb[:],
                                        tags_d[:].rearrange("(a t) -> a t", a=1))
                      tagf = cpool.tile([1, TOK], f16)
                      nc.vector.tensor_copy(tagf[:], tags_sb[:])
                      iotf = iot6
                      oh = cpool.tile([TAGS, TOK], f32)
                      for n in range(NN):
                          tb_ps = ps_gold.tile([TAGS, NW], f32, tag="gemm", bufs=2)
                          nc.tensor.matmul(tb_ps[:], lhsT=ones1[:, 0:TAGS],
                                           rhs=tagf[:, n * NW:(n + 1) * NW],
                                           start=True, stop=True)
                          nc.vector.tensor_tensor(
                              oh[:, n * NW:(n + 1) * NW], tb_ps[:],
                              iotf[:].to_broadcast([TAGS, NW]), op=OP.is_equal)
                      tcol = cpool.tile([TAGS, TOK], f32)
                      for n in range(NN):
                          tc_ps = ps_gold.tile([TAGS, NW], f32, tag="gemm", bufs=2)
                          nc.tensor.matmul(tc_ps[:], lhsT=tr_t[:],
                                           rhs=oh[:, n * NW:(n + 1) * NW],
                                           start=True, stop=True)
                          nc.vector.tensor_copy(tcol[:, n * NW:(n + 1) * NW],
                                                tc_ps[:])
                      gg = cpool.tile([TAGS, TOK], f32)
                      nc.vector.tensor_tensor(gg[:], oh[:], feats[:], op=OP.mult)
                      g2 = cpool.tile([TAGS, TOK], f32)
                      nc.vector.tensor_tensor(g2[:, BL:], oh[:, 0:TOK - BL],
                                              tcol[:, BL:], op=OP.mult)
                      nc.vector.tensor_tensor(
                          g2[:, 0:BL], oh[:, 0:BL],
                          tr_t[:, START:START + 1].to_broadcast([TAGS, BL]),
                          op=OP.mult)
                      gl = cpool.tile([TAGS, BL], f32)
                      nc.vector.tensor_tensor(
                          gl[:], oh[:, TOK - BL:],
                          trT_t[:, STOP:STOP + 1].to_broadcast([TAGS, BL]),
                          op=OP.mult)
                      gold_ps = ps_gold.tile([1, TOK], f32, tag="gold", bufs=1)
                      for n in range(NN):
                          nsl = slice(n * NW, (n + 1) * NW)
                          nc.tensor.matmul(gold_ps[:, nsl], lhsT=ones6[:],
                                           rhs=gg[:, nsl], start=True, stop=False)
                          last = (n == NN - 1)
                          nc.tensor.matmul(gold_ps[:, nsl], lhsT=ones6[:],
                                           rhs=g2[:, nsl], start=False,
                                           stop=not last)
                      nc.tensor.matmul(gold_ps[:, TOK - BL:], lhsT=ones6[:],
                                       rhs=gl[:], start=False, stop=True)
                      gold = pers.tile([1, BL], f32)
                      nc.vector.tensor_reduce(
                          gold[:], gold_ps[:].rearrange("p (t b) -> p b t", b=BL),
                          axis=AX.X, op=OP.add)
                      loss = pers.tile([1, BL], f32)
                      nc.vector.tensor_tensor(loss[:], fs2[:], gold[:],
                                              op=OP.subtract)
                      nc.sync.dma_start(out_d[:], loss[:])

            for _rep in range(reps):
                if _rep:
                    tc.strict_bb_all_engine_barrier()
                emit_pipeline()

    if legalize:
        _legalize_multi_waits(nc)
    return nc


_LW_COUNT = [0]


_SELF_SEM = {mybir.EngineType.DVE: "DVE_", mybir.EngineType.Activation: "Activation_"}


def _legalize_multi_waits(nc):
    """This container's walrus accepts at most ONE sync wait per TPB
    instruction ("Too many sync wait commands" in codegen otherwise), while
    Tile freely attaches several.  Split: keep one wait on the instruction
    and hoist the rest onto standalone InstEventSemaphore instructions on
    the same engine immediately before it (engine-order preserved, so the
    semantics are identical; the stall just happens one slot earlier)."""
    n_new = 0
    for fn in nc.m.functions:
        for bb in fn.blocks:
            out = []
            changed = False
            for inst in bb.instructions:
                si = inst.sync_info
                waits = list(si.on_wait) if si is not None else []
                pref = _SELF_SEM.get(inst.engine)
                if pref is not None and len(waits) > 1:
                    # DVE/ACT are strict in-order single pipelines: a wait on
                    # the engine's own completion semaphore is subsumed by
                    # program order. Drop it (only when other waits remain).
                    kept = [w for w in waits
                            if not str(w.ant_name).startswith(pref)]
                    if kept:
                        waits = kept
                        inst.sync_info = mybir.SyncInfo(
                            on_wait=waits, on_update=list(si.on_update))
                        si = inst.sync_info
                        changed = True
                cap = 1
                if len(waits) > cap:
                    for w in waits[:-cap]:
                        _LW_COUNT[0] += 1
                        es = mybir.InstEventSemaphore(
                            name=f"I-lw{_LW_COUNT[0]}", ins=[], outs=[])
                        es.engine = inst.engine
                        es.sync_info = mybir.SyncInfo(on_wait=[w],
                                                      on_update=[])
                        out.append(es)
                        n_new += 1
                    inst.sync_info = mybir.SyncInfo(
                        on_wait=waits[-cap:], on_update=list(si.on_update))
                    changed = True
                out.append(inst)
            if changed:
                bb.instructions = out
    return n_new


_PROG_CACHE = {}


def _get_program(t_steps=T):
    if t_steps not in _PROG_CACHE:
        _PROG_CACHE[t_steps] = build_program(t_steps)
    return _PROG_CACHE[t_steps]


def _slot_reorder_cols(wT):
    """Reorder the 4H (=1024) columns of [in, 4H] into slot order, and
    pre-scale the g-gate slots by 2 (tanh(g) = 2*sigmoid(2g) - 1, so the
    kernel runs a single Sigmoid over all gates)."""
    chunks = [wT[:, c * 128:(c + 1) * 128].copy() for c in SLOT_SRC]
    chunks[6] = chunks[6] * 2.0
    chunks[7] = chunks[7] * 2.0
    return np.ascontiguousarray(np.concatenate(chunks, axis=1))


def _slot_reorder_vec(v):
    chunks = [v[c * 128:(c + 1) * 128].copy() for c in SLOT_SRC]
    chunks[6] = chunks[6] * 2.0
    chunks[7] = chunks[7] * 2.0
    return np.ascontiguousarray(np.concatenate(chunks))


def prep_inputs(sentences, tags, mask, embed, w_ih, w_hh, b_ih, b_hh,
                aw1, ab1, aw2, ab2, hw, hb, transitions, t_steps=T):
    sent = np.asarray(sentences).astype(np.int32)
    tg = np.asarray(tags).astype(np.int32)
    emb = np.ascontiguousarray(np.asarray(embed, dtype=np.float32))
    w_ih = np.asarray(w_ih, dtype=np.float32)
    w_hh = np.asarray(w_hh, dtype=np.float32)
    b_ih = np.asarray(b_ih, dtype=np.float32)
    b_hh = np.asarray(b_hh, dtype=np.float32)
    trans = np.clip(np.asarray(transitions, dtype=np.float32), -TCLIP, TCLIP)
    wh_np = ml_dtypes.float8_e4m3 if FP8_WHH else np.float16

    shared = {"embed": emb,
              "ident8": np.ascontiguousarray(np.eye(128).astype(wh_np)),
              "aw1T": np.ascontiguousarray(
                  np.asarray(aw1, np.float32).T.astype(np.float16)),
              "ab1": np.ascontiguousarray(np.asarray(ab1, np.float32)),
              "aw2": np.ascontiguousarray(
                  np.asarray(aw2, np.float32).astype(np.float16)),
              "hwT": np.ascontiguousarray(
                  np.asarray(hw, np.float32).T.astype(np.float16)),
              "hb": np.ascontiguousarray(np.asarray(hb, np.float32)),
              "trans": np.ascontiguousarray(trans),
              "transT": np.ascontiguousarray(trans.T)}
    for l in range(3):
        for d in range(2):
            shared[f"wihT_{l}_{d}"] = _slot_reorder_cols(
                w_ih[l, d].T).astype(np.float16)
            shared[f"whhT_{l}_{d}"] = _slot_reorder_cols(
                w_hh[l, d].T).astype(wh_np)
            shared[f"bias_{l}_{d}"] = _slot_reorder_vec(
                b_ih[l, d] + b_hh[l, d])

    in_maps = []
    for c in range(NCORES):
        sh = slice(c * BL, (c + 1) * BL)
        m = dict(shared)
        # (t, b) order, b innermost
        m["tok_ids"] = np.ascontiguousarray(
            sent[sh, :t_steps].T).reshape(-1)
        m["tags_tb"] = np.ascontiguousarray(tg[sh, :t_steps].T).reshape(-1)
        in_maps.append(m)
    return in_maps


def kernel(sentences, tags, mask, embed, w_ih, w_hh, b_ih, b_hh,
           aw1, ab1, aw2, ab2, hw, hb, transitions, _trace=False):
    nc = _get_program()
    in_maps = prep_inputs(sentences, tags, mask, embed, w_ih, w_hh,
                          b_ih, b_hh, aw1, ab1, aw2, ab2, hw, hb,
                          transitions)
    res = run_bass_kernel_spmd(nc, in_maps, core_ids=list(range(NCORES)),
                               trace=_trace)
    parts = np.concatenate([r["loss_part"].reshape(-1) for r in res.results])
    kernel.last_result = res
    return np.float32(parts.mean())
